# revision 11
# baseline (speedup 1.0000x reference)
"""Causal single-head attention (B=4, T=2048, C=H=1024) on 8 TRN2 NeuronCores.

Sharding: core = (batch b, query half qh).  Each core computes attention for
1024 queries of one batch against all 2048 keys of that batch.  The host
passes x ROLLED so the core's own query rows are always rows [0, 1024) of its
input.  In rolled coordinates the causal mask is:
  - keys [0, 1024)   (own half):  triangular mask f >= p  (core independent)
  - keys [1024, 2048) (other half): all-keep or all-drop depending on which
    half this core owns.  Implemented as a per-core bias input (0 or -30)
    added inside the exp activation: exp(s - 30) ~ 5e-13 ~ 0.
Softmax uses no max subtraction (logits are ~N(0, 0.33), |s| < ~2, so exp is
fp32-safe); the denominator is obtained by matmuls against a ones vector and
normalization is reciprocal+multiply.  Fully-masked score tiles are skipped.

Structure (all matmul moving operands 512 wide; f32r ~1cyc/row, LDWEIGHTS
hides under the 213ns stream):
  phase 0: PE-transpose x -> x^T, staged to DRAM (done ONCE; transposes
           don't count as PE activity for the HAM clock gate, so keeping
           them out of the matmul phases keeps the PE at 2.4GHz)
  phase 1-3: pure matmul streams computing Q^T -> DRAM, K^T -> DRAM,
           V -> SBUF resident
  attention: stream Q^T/K^T tiles from DRAM; scores S^T = K^T.T @ Q^T per
           key chunk; exp on ScalarE; triangular mask via affine_select;
           O and denominator accumulate in PSUM; normalize; DMA out.
"""

import math
import sys

sys.path.insert(0, "/opt/trn_rl_repo")

import numpy as np

B, T, C, H = 4, 2048, 1024, 1024
NCORES = 8
TQ = T // 2          # queries per core
P = 128              # partitions
CC = C // P          # contraction chunks for projections
HC = H // P          # contraction chunks for scores
NTK = T // P         # key chunks (16)
TQT = 512            # tq tile width in attention phase
NJ = TQ // TQT       # 2 tq tiles
NQC = TQT // P       # 4 query chunks of 128 per tq tile
XBLK = 512           # x^T columns per projection block
SCALE = 1.0 / math.sqrt(H)
TAIL_BIAS = -30.0

MM_DTYPE = "float32r"


def _active_tkcs(j):
    """Key chunks contributing to tq tile j (rolled coords)."""
    nblk = TQT // P
    return [
        tkc for tkc in range(NTK) if tkc >= NTK // 2 or tkc < nblk * (j + 1)
    ]


def _attn_body(tc, out_ap, xr, wq, wk, wv, tailbias):
    import concourse.mybir as mybir
    from concourse.masks import make_identity

    nc = tc.nc
    f32 = mybir.dt.float32
    mmdt = getattr(mybir.dt, MM_DTYPE)

    from contextlib import ExitStack

    with ExitStack() as ctx:
        consts = ctx.enter_context(tc.tile_pool(name="consts", bufs=1))
        ones_f32 = consts.tile([P, 2], f32)
        nc.vector.memset(ones_f32, 1.0)
        ones = consts.tile([P, 2], mmdt)
        nc.vector.tensor_copy(ones, ones_f32)
        tail_sb = consts.tile([P, 1], f32)
        nc.sync.dma_start(out=tail_sb, in_=tailbias)
        warm = consts.tile([P, 1], f32)
        nc.scalar.activation(warm, tail_sb, mybir.ActivationFunctionType.Exp)

        big = ctx.enter_context(tc.tile_pool(name="big", bufs=1))
        V = big.tile([P, NTK, H], mmdt)    # V: [tk, h], 64KB/part resident

        dram = ctx.enter_context(
            tc.tile_pool(name="dram", bufs=1, space="DRAM")
        )
        NTT = T // XBLK
        XTd = dram.tile([NTT, CC, P, XBLK], mmdt)   # x^T blocks
        QTd = dram.tile([HC, P, TQ], mmdt)          # Q^T: [hc, h, tq]
        KTd = dram.tile([NTK, HC, P, P], mmdt)      # K^T: [tkc][hc, h, tk]

        # ---------------- phase 0: transpose x -> x^T -> DRAM ----------------
        with ExitStack() as pctx:
            idp = pctx.enter_context(tc.tile_pool(name="idp", bufs=1))
            xrow_pool = pctx.enter_context(tc.tile_pool(name="xrow", bufs=3))
            xtc_pool = pctx.enter_context(tc.tile_pool(name="xtc", bufs=3))
            tpsum = pctx.enter_context(
                tc.tile_pool(name="tpsum", bufs=8, space="PSUM")
            )
            identity = idp.tile([P, P], f32)
            make_identity(nc, identity)
            for tch in range(NTK):
                xrow = xrow_pool.tile([P, C], f32, tag="xr")
                nc.sync.dma_start(out=xrow, in_=xr[tch * P : (tch + 1) * P, :])
                xtc = xtc_pool.tile([P, CC, P], mmdt, tag="xtc")
                for cc in range(CC):
                    pt = tpsum.tile([P, P], f32, tag="tp")
                    nc.tensor.transpose(
                        pt, xrow[:, cc * P : (cc + 1) * P], identity
                    )
                    nc.any.tensor_copy(xtc[:, cc, :], pt)
                nc.sync.dma_start(
                    out=XTd[
                        tch // (XBLK // P),
                        :,
                        :,
                        (tch % (XBLK // P)) * P : (tch % (XBLK // P) + 1) * P,
                    ].rearrange("cc p t -> p cc t"),
                    in_=xtc,
                )

        # ---------------- phases 1-3: projections (pure matmul) -------------
        with ExitStack() as pctx:
            wpool = pctx.enter_context(tc.tile_pool(name="wpool", bufs=2))
            xt_pool = pctx.enter_context(tc.tile_pool(name="xt", bufs=2))
            ppsum = pctx.enter_context(
                tc.tile_pool(name="ppsum", bufs=6, space="PSUM")
            )
            stage = pctx.enter_context(tc.tile_pool(name="pstage", bufs=3))

            def load_w(w_dram, name):
                w_sb = wpool.tile([P, CC, H], mmdt, tag="w", name=name)
                wr = w_dram.rearrange("(cc p) h -> p cc h", p=P)
                nc.sync.dma_start(out=w_sb[:, :, 0:512], in_=wr[:, :, 0:512])
                nc.sync.dma_start(out=w_sb[:, :, 512:H], in_=wr[:, :, 512:H])
                return w_sb

            def load_xt(tt, name):
                xt = xt_pool.tile([P, CC, XBLK], mmdt, tag="xt", name=name)
                nc.sync.dma_start(
                    out=xt, in_=XTd[tt].rearrange("cc p t -> p cc t")
                )
                return xt

            # Q^T (query rows = blocks 0..1) -> QTd
            w_sb = load_w(wq, "wq_sb")
            for tt in range(TQ // XBLK):
                xt = load_xt(tt, "xt_q")
                for hc in range(HC):
                    ps = ppsum.tile([P, XBLK], f32, tag="pp")
                    for cc in range(CC):
                        nc.tensor.matmul(
                            ps,
                            w_sb[:, cc, hc * P : (hc + 1) * P],
                            xt[:, cc, :],
                            start=(cc == 0),
                            stop=(cc == CC - 1),
                        )
                    st = stage.tile([P, XBLK], mmdt, tag="st")
                    nc.any.tensor_copy(st, ps)
                    nc.sync.dma_start(
                        out=QTd[hc, :, tt * XBLK : (tt + 1) * XBLK], in_=st
                    )

            # K^T -> KTd
            w_sb = load_w(wk, "wk_sb")
            for tt in range(NTT):
                xt = load_xt(tt, "xt_k")
                for hc in range(HC):
                    ps = ppsum.tile([P, XBLK], f32, tag="pp")
                    for cc in range(CC):
                        nc.tensor.matmul(
                            ps,
                            w_sb[:, cc, hc * P : (hc + 1) * P],
                            xt[:, cc, :],
                            start=(cc == 0),
                            stop=(cc == CC - 1),
                        )
                    st = stage.tile([P, XBLK], mmdt, tag="st")
                    nc.any.tensor_copy(st, ps)
                    for sub in range(XBLK // P):
                        nc.sync.dma_start(
                            out=KTd[tt * (XBLK // P) + sub, hc, :, :],
                            in_=st[:, sub * P : (sub + 1) * P],
                        )

            # V -> SBUF resident
            w_sb = load_w(wv, "wv_sb")
            for tt in range(NTT):
                xt = load_xt(tt, "xt_v")
                for sub in range(XBLK // P):
                    tkc = tt * (XBLK // P) + sub
                    for ht in range(H // 512):
                        ps = ppsum.tile([P, 512], f32, tag="pp")
                        for cc in range(CC):
                            nc.tensor.matmul(
                                ps,
                                xt[:, cc, sub * P : (sub + 1) * P],
                                w_sb[:, cc, ht * 512 : (ht + 1) * 512],
                                start=(cc == 0),
                                stop=(cc == CC - 1),
                            )
                        nc.any.tensor_copy(
                            V[:, tkc, ht * 512 : (ht + 1) * 512], ps
                        )

        # ---------------- attention phase ----------------
        with ExitStack() as actx:
            qt_pool = actx.enter_context(tc.tile_pool(name="qt", bufs=2))
            kt_pool = actx.enter_context(tc.tile_pool(name="kt", bufs=4))
            et_pool = actx.enter_context(tc.tile_pool(name="et", bufs=2))
            spsum = actx.enter_context(
                tc.tile_pool(name="spsum", bufs=2, space="PSUM")
            )
            opsum = actx.enter_context(
                tc.tile_pool(name="opsum", bufs=4, space="PSUM")
            )
            dpsum = actx.enter_context(
                tc.tile_pool(name="dpsum", bufs=2, space="PSUM")
            )
            small = actx.enter_context(tc.tile_pool(name="small", bufs=4))
            ostage = actx.enter_context(tc.tile_pool(name="ostage", bufs=4))

            for j in range(NJ):
                q0 = j * TQT
                qt = qt_pool.tile([P, HC, TQT], mmdt, tag="qt")
                nc.sync.dma_start(
                    out=qt,
                    in_=QTd[:, :, q0 : q0 + TQT].rearrange("hc p t -> p hc t"),
                )
                et = et_pool.tile([P, NTK, TQT], mmdt, tag="et")
                actives = _active_tkcs(j)
                # all 4 per-qc denominators share one PSUM bank
                d_ps = dpsum.tile([P, NQC, 2], f32, tag="dp")

                # scores + exp (+ mask) for every active key chunk
                for tkc in actives:
                    kt = kt_pool.tile([P, HC, P], mmdt, tag="kt",
                                      name=f"kt_{j}_{tkc}")
                    nc.sync.dma_start(
                        out=kt, in_=KTd[tkc].rearrange("hc p t -> p hc t")
                    )
                    sp = spsum.tile([P, TQT], f32, tag="sp", name=f"sp_{tkc}")
                    for hc in range(HC):
                        nc.tensor.matmul(
                            sp,
                            kt[:, hc, :],
                            qt[:, hc, :],
                            start=(hc == 0),
                            stop=(hc == HC - 1),
                        )
                    bias = tail_sb if tkc >= NTK // 2 else 0.0
                    nc.scalar.activation(
                        et[:, tkc, :],
                        sp,
                        mybir.ActivationFunctionType.Exp,
                        bias=bias,
                        scale=SCALE,
                    )
                    # diagonal-crossing tiles: triangular mask in rolled coords
                    if TQT * j <= tkc * P < TQT * (j + 1):
                        nc.gpsimd.affine_select(
                            out=et[:, tkc, :],
                            in_=et[:, tkc, :],
                            compare_op=mybir.AluOpType.is_ge,
                            fill=0.0,
                            base=TQT * j - P * tkc,
                            pattern=[[1, TQT]],
                            channel_multiplier=-1,
                        )
                    # denominator accumulation (one bank, single group)
                    for qc in range(NQC):
                        nc.tensor.matmul(
                            d_ps[:, qc, :],
                            et[:, tkc, qc * P : (qc + 1) * P],
                            ones,
                            start=(tkc == actives[0] and qc == 0),
                            stop=(tkc == actives[-1] and qc == NQC - 1),
                        )

                # O accumulation: one ht at a time so PSUM fits (4 qc tiles)
                for ht in range(2):
                    o_ps = [
                        opsum.tile([P, 512], f32, tag="op", name=f"o_{qc}")
                        for qc in range(NQC)
                    ]
                    for i, tkc in enumerate(actives):
                        for qc in range(NQC):
                            nc.tensor.matmul(
                                o_ps[qc],
                                et[:, tkc, qc * P : (qc + 1) * P],
                                V[:, tkc, ht * 512 : (ht + 1) * 512],
                                start=(i == 0),
                                stop=(i == len(actives) - 1),
                            )
                    for qc in range(NQC):
                        rec = small.tile([P, 1], f32, tag="rec")
                        nc.vector.reciprocal(rec, d_ps[:, qc, 0:1])
                        ot = ostage.tile([P, 512], f32, tag="ot")
                        nc.vector.tensor_scalar_mul(ot, o_ps[qc], rec)
                        nc.sync.dma_start(
                            out=out_ap[
                                q0 + qc * P : q0 + (qc + 1) * P,
                                ht * 512 : (ht + 1) * 512,
                            ],
                            in_=ot,
                        )


def build_nc():
    import concourse.mybir as mybir
    import concourse.tile as tile
    from concourse import bacc

    nc = bacc.Bacc(
        "TRN2",
        target_bir_lowering=False,
        debug=False,
        num_devices=NCORES,
    )
    f32 = mybir.dt.float32
    mmdt = getattr(mybir.dt, MM_DTYPE)
    xr = nc.dram_tensor("xr", [T, C], f32, kind="ExternalInput").ap()
    wq = nc.dram_tensor("wq", [C, H], mmdt, kind="ExternalInput").ap()
    wk = nc.dram_tensor("wk", [C, H], mmdt, kind="ExternalInput").ap()
    wv = nc.dram_tensor("wv", [C, H], mmdt, kind="ExternalInput").ap()
    tailbias = nc.dram_tensor(
        "tailbias", [P, 1], f32, kind="ExternalInput"
    ).ap()
    out = nc.dram_tensor("out", [TQ, H], f32, kind="ExternalOutput").ap()
    with tile.TileContext(nc) as tc:
        _attn_body(tc, out, xr, wq, wk, wv, tailbias)
    nc.compile()
    return nc


def make_in_maps(x, Wk, Wq, Wv):
    x = np.asarray(x, dtype=np.float32)
    Wk = np.ascontiguousarray(np.asarray(Wk, dtype=np.float32))
    Wq = np.ascontiguousarray(np.asarray(Wq, dtype=np.float32))
    Wv = np.ascontiguousarray(np.asarray(Wv, dtype=np.float32))
    in_maps = []
    for c in range(NCORES):
        b, qh = divmod(c, 2)
        xb = x[b]
        if qh == 0:
            xr = np.ascontiguousarray(xb)
            tail = np.full((P, 1), TAIL_BIAS, dtype=np.float32)
        else:
            xr = np.ascontiguousarray(
                np.concatenate([xb[TQ:], xb[:TQ]], axis=0)
            )
            tail = np.zeros((P, 1), dtype=np.float32)
        in_maps.append(
            {"xr": xr, "wq": Wq, "wk": Wk, "wv": Wv, "tailbias": tail}
        )
    return in_maps


def assemble_out(results):
    out = np.empty((B, T, H), dtype=np.float32)
    for c in range(NCORES):
        b, qh = divmod(c, 2)
        out[b, qh * TQ : (qh + 1) * TQ] = results[c]["out"]
    return out


def kernel(x, Wk, Wq, Wv):
    from concourse import bass_utils

    nc = build_nc()
    in_maps = make_in_maps(x, Wk, Wq, Wv)
    res = bass_utils.run_bass_kernel_spmd(
        nc, in_maps, core_ids=list(range(NCORES))
    )
    return assemble_out(res.results)


# revision 12
# speedup vs baseline: 1.1667x; 1.1667x over previous
"""Causal single-head attention (B=4, T=2048, C=H=1024) on 8 TRN2 NeuronCores.

Sharding: core = (batch b, query half qh).  Each core computes attention for
1024 queries of one batch against all 2048 keys of that batch.  The host
passes x ROLLED so the core's own query rows are always rows [0, 1024) of its
input.  In rolled coordinates the causal mask is:
  - keys [0, 1024)   (own half):  triangular mask f >= p  (core independent)
  - keys [1024, 2048) (other half): all-keep or all-drop depending on which
    half this core owns.  Implemented as a per-core bias input (0 or -30)
    added inside the exp activation: exp(s - 30) ~ 5e-13 ~ 0.
Softmax uses no max subtraction (logits are ~N(0, 0.33), |s| < ~2, so exp is
fp32-safe); the denominator is obtained by matmuls against a ones vector and
normalization is reciprocal+multiply.  Fully-masked score tiles are skipped.

Structure (all matmul moving operands 512 wide; f32r ~1cyc/row, LDWEIGHTS
hides under the 213ns stream):
  phase 0: PE-transpose x -> x^T, staged to DRAM in 512-col blocks (done
           ONCE; transposes don't count as PE activity for the HAM clock
           gate, so keeping them out of the matmul phases keeps 2.4GHz)
  phase 1-3: pure matmul streams: Q^T -> DRAM, K^T -> DRAM, V -> SBUF
  attention: stream Q^T/K^T from DRAM; scores S^T = K^T.T @ Q^T per key
           chunk; exp on ScalarE; triangular mask via affine_select; O and
           denominator accumulate in PSUM; normalize; DMA out.
DMA queues: loads on Sync (HWDGE), stores on GpSimd (SWDGE) so store bursts
never head-of-line-block the loads the PE is waiting for.  DRAM staging
layouts are chosen so every DMA runs >=2KB contiguous per descriptor, and
x^T staging is one DRAM tile per block so later phases start as soon as
their block is written.
"""

import math
import sys

sys.path.insert(0, "/opt/trn_rl_repo")

import numpy as np

B, T, C, H = 4, 2048, 1024, 1024
NCORES = 8
TQ = T // 2          # queries per core
P = 128              # partitions
CC = C // P          # contraction chunks for projections
HC = H // P          # contraction chunks for scores
NTK = T // P         # key chunks (16)
TQT = 512            # tq tile width in attention phase
NJ = TQ // TQT       # 2 tq tiles
NQC = TQT // P       # 4 query chunks of 128 per tq tile
XBLK = 512           # x^T columns per block
NTT = T // XBLK      # 4 blocks
SCALE = 1.0 / math.sqrt(H)
TAIL_BIAS = -30.0

MM_DTYPE = "float32r"


def _active_tkcs(j):
    """Key chunks contributing to tq tile j (rolled coords)."""
    nblk = TQT // P
    return [
        tkc for tkc in range(NTK) if tkc >= NTK // 2 or tkc < nblk * (j + 1)
    ]


def _attn_body(tc, out_ap, xr, wq, wk, wv, tailbias):
    import concourse.mybir as mybir
    from concourse.masks import make_identity

    nc = tc.nc
    f32 = mybir.dt.float32
    mmdt = getattr(mybir.dt, MM_DTYPE)

    from contextlib import ExitStack

    with ExitStack() as ctx:
        consts = ctx.enter_context(tc.tile_pool(name="consts", bufs=1))
        ones_f32 = consts.tile([P, 2], f32)
        nc.vector.memset(ones_f32, 1.0)
        ones = consts.tile([P, 2], mmdt)
        nc.vector.tensor_copy(ones, ones_f32)
        tail_sb = consts.tile([P, 1], f32)
        nc.sync.dma_start(out=tail_sb, in_=tailbias)
        warm = consts.tile([P, 1], f32)
        nc.scalar.activation(warm, tail_sb, mybir.ActivationFunctionType.Exp)

        big = ctx.enter_context(tc.tile_pool(name="big", bufs=1))
        V = big.tile([P, NTK, H], mmdt)    # V: [tk, h], 64KB/part resident

        dram = ctx.enter_context(
            tc.tile_pool(name="dram", bufs=1, space="DRAM")
        )
        # x^T: one tile per 512-col block, [p][cc][t] contiguous per partition
        XTd = [
            dram.tile([P, CC, XBLK], mmdt, name=f"xtd_{tt}")
            for tt in range(NTT)
        ]
        QTd = dram.tile([HC, P, TQ], mmdt)   # Q^T: [hc, h, tq]
        KTd = dram.tile([HC, P, T], mmdt)    # K^T: [hc, h, tk]

        # ---------------- phase 0: transpose x -> x^T -> DRAM ----------------
        with ExitStack() as pctx:
            idp = pctx.enter_context(tc.tile_pool(name="idp", bufs=1))
            xrow_pool = pctx.enter_context(tc.tile_pool(name="xrow", bufs=3))
            xtb_pool = pctx.enter_context(tc.tile_pool(name="xtb", bufs=2))
            tpsum = pctx.enter_context(
                tc.tile_pool(name="tpsum", bufs=8, space="PSUM")
            )
            identity = idp.tile([P, P], f32)
            make_identity(nc, identity)
            for tt in range(NTT):
                xtb = xtb_pool.tile([P, CC, XBLK], mmdt, tag="xtb")
                for sub in range(XBLK // P):
                    tch = tt * (XBLK // P) + sub
                    xrow = xrow_pool.tile([P, C], f32, tag="xr")
                    nc.sync.dma_start(
                        out=xrow, in_=xr[tch * P : (tch + 1) * P, :]
                    )
                    for cc in range(CC):
                        pt = tpsum.tile([P, P], f32, tag="tp")
                        nc.tensor.transpose(
                            pt, xrow[:, cc * P : (cc + 1) * P], identity
                        )
                        nc.any.tensor_copy(
                            xtb[:, cc, sub * P : (sub + 1) * P], pt
                        )
                nc.gpsimd.dma_start(out=XTd[tt][:], in_=xtb)

        # ---------------- phases 1-3: projections (pure matmul) -------------
        with ExitStack() as pctx:
            wpool = pctx.enter_context(tc.tile_pool(name="wpool", bufs=2))
            xt_pool = pctx.enter_context(tc.tile_pool(name="xt", bufs=2))
            ppsum = pctx.enter_context(
                tc.tile_pool(name="ppsum", bufs=6, space="PSUM")
            )
            stage = pctx.enter_context(tc.tile_pool(name="pstage", bufs=3))

            def load_w(w_dram, name):
                w_sb = wpool.tile([P, CC, H], mmdt, tag="w", name=name)
                wr = w_dram.rearrange("(cc p) h -> p cc h", p=P)
                nc.sync.dma_start(out=w_sb[:, :, 0:512], in_=wr[:, :, 0:512])
                nc.sync.dma_start(out=w_sb[:, :, 512:H], in_=wr[:, :, 512:H])
                return w_sb

            def load_xt(tt, name):
                xt = xt_pool.tile([P, CC, XBLK], mmdt, tag="xt", name=name)
                nc.sync.dma_start(out=xt, in_=XTd[tt][:])
                return xt

            # Q^T (query rows = blocks 0..1) -> QTd
            w_sb = load_w(wq, "wq_sb")
            for tt in range(TQ // XBLK):
                xt = load_xt(tt, "xt_q")
                for hc in range(HC):
                    ps = ppsum.tile([P, XBLK], f32, tag="pp")
                    for cc in range(CC):
                        nc.tensor.matmul(
                            ps,
                            w_sb[:, cc, hc * P : (hc + 1) * P],
                            xt[:, cc, :],
                            start=(cc == 0),
                            stop=(cc == CC - 1),
                        )
                    st = stage.tile([P, XBLK], mmdt, tag="st")
                    nc.any.tensor_copy(st, ps)
                    nc.gpsimd.dma_start(
                        out=QTd[hc, :, tt * XBLK : (tt + 1) * XBLK], in_=st
                    )

            # K^T -> KTd
            w_sb = load_w(wk, "wk_sb")
            for tt in range(NTT):
                xt = load_xt(tt, "xt_k")
                for hc in range(HC):
                    ps = ppsum.tile([P, XBLK], f32, tag="pp")
                    for cc in range(CC):
                        nc.tensor.matmul(
                            ps,
                            w_sb[:, cc, hc * P : (hc + 1) * P],
                            xt[:, cc, :],
                            start=(cc == 0),
                            stop=(cc == CC - 1),
                        )
                    st = stage.tile([P, XBLK], mmdt, tag="st")
                    nc.any.tensor_copy(st, ps)
                    nc.gpsimd.dma_start(
                        out=KTd[hc, :, tt * XBLK : (tt + 1) * XBLK], in_=st
                    )

            # V -> SBUF resident
            w_sb = load_w(wv, "wv_sb")
            for tt in range(NTT):
                xt = load_xt(tt, "xt_v")
                for sub in range(XBLK // P):
                    tkc = tt * (XBLK // P) + sub
                    for ht in range(H // 512):
                        ps = ppsum.tile([P, 512], f32, tag="pp")
                        for cc in range(CC):
                            nc.tensor.matmul(
                                ps,
                                xt[:, cc, sub * P : (sub + 1) * P],
                                w_sb[:, cc, ht * 512 : (ht + 1) * 512],
                                start=(cc == 0),
                                stop=(cc == CC - 1),
                            )
                        nc.any.tensor_copy(
                            V[:, tkc, ht * 512 : (ht + 1) * 512], ps
                        )

        # ---------------- attention phase ----------------
        with ExitStack() as actx:
            qt_pool = actx.enter_context(tc.tile_pool(name="qt", bufs=1))
            kt_pool = actx.enter_context(tc.tile_pool(name="kt", bufs=2))
            et_pool = actx.enter_context(tc.tile_pool(name="et", bufs=2))
            spsum = actx.enter_context(
                tc.tile_pool(name="spsum", bufs=2, space="PSUM")
            )
            opsum = actx.enter_context(
                tc.tile_pool(name="opsum", bufs=4, space="PSUM")
            )
            dpsum = actx.enter_context(
                tc.tile_pool(name="dpsum", bufs=2, space="PSUM")
            )
            small = actx.enter_context(tc.tile_pool(name="small", bufs=4))
            ostage = actx.enter_context(tc.tile_pool(name="ostage", bufs=4))

            for j in range(NJ):
                q0 = j * TQT
                qt = qt_pool.tile([P, HC, TQT], mmdt, tag="qt")
                nc.sync.dma_start(
                    out=qt,
                    in_=QTd[:, :, q0 : q0 + TQT].rearrange("hc p t -> p hc t"),
                )
                et = et_pool.tile([P, NTK, TQT], mmdt, tag="et")
                actives = _active_tkcs(j)
                tts = sorted(set(tkc // (XBLK // P) for tkc in actives))
                kts = {}
                for tt in tts:
                    kt = kt_pool.tile(
                        [P, HC, XBLK], mmdt, tag="kt", name=f"kt_{j}_{tt}"
                    )
                    nc.sync.dma_start(
                        out=kt,
                        in_=KTd[:, :, tt * XBLK : (tt + 1) * XBLK].rearrange(
                            "hc p t -> p hc t"
                        ),
                    )
                    kts[tt] = kt
                # all 4 per-qc denominators share one PSUM bank
                d_ps = dpsum.tile([P, NQC, 2], f32, tag="dp")

                # scores + exp (+ mask) for every active key chunk
                for tkc in actives:
                    kt = kts[tkc // (XBLK // P)]
                    k0 = (tkc % (XBLK // P)) * P
                    sp = spsum.tile([P, TQT], f32, tag="sp", name=f"sp_{tkc}")
                    for hc in range(HC):
                        nc.tensor.matmul(
                            sp,
                            kt[:, hc, k0 : k0 + P],
                            qt[:, hc, :],
                            start=(hc == 0),
                            stop=(hc == HC - 1),
                        )
                    bias = tail_sb if tkc >= NTK // 2 else 0.0
                    nc.scalar.activation(
                        et[:, tkc, :],
                        sp,
                        mybir.ActivationFunctionType.Exp,
                        bias=bias,
                        scale=SCALE,
                    )
                    # diagonal-crossing tiles: triangular mask in rolled coords
                    if TQT * j <= tkc * P < TQT * (j + 1):
                        nc.gpsimd.affine_select(
                            out=et[:, tkc, :],
                            in_=et[:, tkc, :],
                            compare_op=mybir.AluOpType.is_ge,
                            fill=0.0,
                            base=TQT * j - P * tkc,
                            pattern=[[1, TQT]],
                            channel_multiplier=-1,
                        )
                    # denominator accumulation (one bank, single group)
                    for qc in range(NQC):
                        nc.tensor.matmul(
                            d_ps[:, qc, :],
                            et[:, tkc, qc * P : (qc + 1) * P],
                            ones,
                            start=(tkc == actives[0] and qc == 0),
                            stop=(tkc == actives[-1] and qc == NQC - 1),
                        )

                # O accumulation: one ht at a time so PSUM fits (4 qc tiles)
                for ht in range(2):
                    o_ps = [
                        opsum.tile([P, 512], f32, tag="op", name=f"o_{qc}")
                        for qc in range(NQC)
                    ]
                    for i, tkc in enumerate(actives):
                        for qc in range(NQC):
                            nc.tensor.matmul(
                                o_ps[qc],
                                et[:, tkc, qc * P : (qc + 1) * P],
                                V[:, tkc, ht * 512 : (ht + 1) * 512],
                                start=(i == 0),
                                stop=(i == len(actives) - 1),
                            )
                    for qc in range(NQC):
                        rec = small.tile([P, 1], f32, tag="rec")
                        nc.vector.reciprocal(rec, d_ps[:, qc, 0:1])
                        ot = ostage.tile([P, 512], f32, tag="ot")
                        nc.vector.tensor_scalar_mul(ot, o_ps[qc], rec)
                        nc.gpsimd.dma_start(
                            out=out_ap[
                                q0 + qc * P : q0 + (qc + 1) * P,
                                ht * 512 : (ht + 1) * 512,
                            ],
                            in_=ot,
                        )


def build_nc():
    import concourse.mybir as mybir
    import concourse.tile as tile
    from concourse import bacc

    nc = bacc.Bacc(
        "TRN2",
        target_bir_lowering=False,
        debug=False,
        num_devices=NCORES,
    )
    f32 = mybir.dt.float32
    mmdt = getattr(mybir.dt, MM_DTYPE)
    xr = nc.dram_tensor("xr", [T, C], f32, kind="ExternalInput").ap()
    wq = nc.dram_tensor("wq", [C, H], mmdt, kind="ExternalInput").ap()
    wk = nc.dram_tensor("wk", [C, H], mmdt, kind="ExternalInput").ap()
    wv = nc.dram_tensor("wv", [C, H], mmdt, kind="ExternalInput").ap()
    tailbias = nc.dram_tensor(
        "tailbias", [P, 1], f32, kind="ExternalInput"
    ).ap()
    out = nc.dram_tensor("out", [TQ, H], f32, kind="ExternalOutput").ap()
    with tile.TileContext(nc) as tc:
        _attn_body(tc, out, xr, wq, wk, wv, tailbias)
    nc.compile()
    return nc


def make_in_maps(x, Wk, Wq, Wv):
    x = np.asarray(x, dtype=np.float32)
    Wk = np.ascontiguousarray(np.asarray(Wk, dtype=np.float32))
    Wq = np.ascontiguousarray(np.asarray(Wq, dtype=np.float32))
    Wv = np.ascontiguousarray(np.asarray(Wv, dtype=np.float32))
    in_maps = []
    for c in range(NCORES):
        b, qh = divmod(c, 2)
        xb = x[b]
        if qh == 0:
            xr = np.ascontiguousarray(xb)
            tail = np.full((P, 1), TAIL_BIAS, dtype=np.float32)
        else:
            xr = np.ascontiguousarray(
                np.concatenate([xb[TQ:], xb[:TQ]], axis=0)
            )
            tail = np.zeros((P, 1), dtype=np.float32)
        in_maps.append(
            {"xr": xr, "wq": Wq, "wk": Wk, "wv": Wv, "tailbias": tail}
        )
    return in_maps


def assemble_out(results):
    out = np.empty((B, T, H), dtype=np.float32)
    for c in range(NCORES):
        b, qh = divmod(c, 2)
        out[b, qh * TQ : (qh + 1) * TQ] = results[c]["out"]
    return out


def kernel(x, Wk, Wq, Wv):
    from concourse import bass_utils

    nc = build_nc()
    in_maps = make_in_maps(x, Wk, Wq, Wv)
    res = bass_utils.run_bass_kernel_spmd(
        nc, in_maps, core_ids=list(range(NCORES))
    )
    return assemble_out(res.results)


# revision 14
# speedup vs baseline: 1.1856x; 1.0162x over previous
"""Causal single-head attention (B=4, T=2048, C=H=1024) on 8 TRN2 NeuronCores.

Sharding: core = (batch b, query half qh).  Each core computes attention for
1024 queries of one batch against all 2048 keys of that batch.  The host
passes x ROLLED so the core's own query rows are always rows [0, 1024) of its
input.  In rolled coordinates the causal mask is:
  - keys [0, 1024)   (own half):  triangular mask f >= p  (core independent)
  - keys [1024, 2048) (other half): all-keep or all-drop depending on which
    half this core owns.  Implemented as a per-core bias input (0 or -30)
    added inside the exp activation: exp(s - 30) ~ 5e-13 ~ 0.
Softmax uses no max subtraction (logits are ~N(0, 0.33), |s| < ~2, so exp is
fp32-safe); the denominator is obtained by matmuls against a ones vector and
normalization is reciprocal+multiply.  Fully-masked score tiles are skipped.

Structure (all matmul moving operands 512 wide; f32r ~1cyc/row, LDWEIGHTS
hides under the 213ns stream):
  phase 0: PE-transpose x -> x^T, staged to DRAM in 512-col blocks (done
           ONCE; transposes don't count as PE activity for the HAM clock
           gate, so keeping them out of the matmul phases keeps 2.4GHz).
           4 transposes share one PSUM bank -> one 512-wide evacuation.
  phase 1-3: pure matmul streams: Q^T -> DRAM, K^T -> DRAM, V -> SBUF
  attention: stream Q^T/K^T from DRAM; scores S^T = K^T.T @ Q^T per key
           chunk; exp on ScalarE; triangular mask via affine_select; O and
           denominator accumulate in PSUM; normalize; DMA out.
All 16KB [P, 8, 512] staging tiles (x^T blocks, q^T, k^T spans) share ONE
pool tag so there are no pool-scope barriers between phases and prefetches
cross phase boundaries.  Loads go on Sync (HWDGE), stores on GpSimd (SWDGE).
Weight tiles double-buffer from kernel start.
"""

import math
import sys

sys.path.insert(0, "/opt/trn_rl_repo")

import numpy as np

B, T, C, H = 4, 2048, 1024, 1024
NCORES = 8
TQ = T // 2          # queries per core
P = 128              # partitions
CC = C // P          # contraction chunks for projections
HC = H // P          # contraction chunks for scores
NTK = T // P         # key chunks (16)
TQT = 512            # tq tile width in attention phase
NJ = TQ // TQT       # 2 tq tiles
NQC = TQT // P       # 4 query chunks of 128 per tq tile
XBLK = 512           # x^T columns per block
NTT = T // XBLK      # 4 blocks
SCALE = 1.0 / math.sqrt(H)
TAIL_BIAS = -30.0

MM_DTYPE = "float32r"


def _active_tkcs(j):
    """Key chunks contributing to tq tile j (rolled coords)."""
    nblk = TQT // P
    return [
        tkc for tkc in range(NTK) if tkc >= NTK // 2 or tkc < nblk * (j + 1)
    ]


def _attn_body(tc, out_ap, xr, wq, wk, wv, tailbias):
    import concourse.mybir as mybir
    from concourse.masks import make_identity

    nc = tc.nc
    f32 = mybir.dt.float32
    mmdt = getattr(mybir.dt, MM_DTYPE)

    from contextlib import ExitStack

    with ExitStack() as ctx:
        consts = ctx.enter_context(tc.tile_pool(name="consts", bufs=1))
        ones_f32 = consts.tile([P, 2], f32)
        nc.vector.memset(ones_f32, 1.0)
        ones = consts.tile([P, 2], mmdt)
        nc.vector.tensor_copy(ones, ones_f32)
        tail_sb = consts.tile([P, 1], f32)
        nc.sync.dma_start(out=tail_sb, in_=tailbias)
        warm = consts.tile([P, 1], f32)
        nc.scalar.activation(warm, tail_sb, mybir.ActivationFunctionType.Exp)

        big = ctx.enter_context(tc.tile_pool(name="big", bufs=1))
        V = big.tile([P, NTK, H], mmdt)    # V: [tk, h], 64KB/part resident

        # [P, 8, 512] staging tiles: x^T blocks, q^T, k^T spans — one tag
        blk_pool = ctx.enter_context(tc.tile_pool(name="blk", bufs=3))

        def blk_tile(name):
            return blk_pool.tile([P, 8, XBLK], mmdt, tag="blk", name=name)

        dram = ctx.enter_context(
            tc.tile_pool(name="dram", bufs=1, space="DRAM")
        )
        # x^T: one tile per 512-col block, [p][cc][t] contiguous per partition
        XTd = [
            dram.tile([P, CC, XBLK], mmdt, name=f"xtd_{tt}")
            for tt in range(NTT)
        ]
        QTd = dram.tile([HC, P, TQ], mmdt)   # Q^T: [hc, h, tq]
        KTd = dram.tile([HC, P, T], mmdt)    # K^T: [hc, h, tk]

        # ------ phases 0-3: transpose + projections (shared W pool) --------
        with ExitStack() as pctx:
            wpool = pctx.enter_context(tc.tile_pool(name="wpool", bufs=2))
            idp = pctx.enter_context(tc.tile_pool(name="idp", bufs=1))
            xrow_pool = pctx.enter_context(tc.tile_pool(name="xrow", bufs=2))
            tpsum = pctx.enter_context(
                tc.tile_pool(name="tpsum", bufs=2, space="PSUM")
            )
            ppsum = pctx.enter_context(
                tc.tile_pool(name="ppsum", bufs=6, space="PSUM")
            )
            stage = pctx.enter_context(tc.tile_pool(name="pstage", bufs=3))

            def load_w(w_dram, name):
                w_sb = wpool.tile([P, CC, H], mmdt, tag="w", name=name)
                wr = w_dram.rearrange("(cc p) h -> p cc h", p=P)
                nc.sync.dma_start(out=w_sb[:, :, 0:512], in_=wr[:, :, 0:512])
                nc.sync.dma_start(out=w_sb[:, :, 512:H], in_=wr[:, :, 512:H])
                return w_sb

            w_q = load_w(wq, "wq_sb")  # prefetch from kernel start
            identity = idp.tile([P, P], f32)
            make_identity(nc, identity)

            # phase 0: transpose x -> x^T -> DRAM
            for tt in range(NTT):
                xtb = blk_tile(f"xtb_{tt}")
                for sub in range(XBLK // P):
                    tch = tt * (XBLK // P) + sub
                    xrow = xrow_pool.tile([P, C], f32, tag="xr")
                    nc.sync.dma_start(
                        out=xrow, in_=xr[tch * P : (tch + 1) * P, :]
                    )
                    for g in range(2):  # 4 transposes share one PSUM bank
                        pt = tpsum.tile([P, 4, P], f32, tag="tp")
                        for q in range(4):
                            cc = g * 4 + q
                            nc.tensor.matmul(
                                pt[:, q, :],
                                xrow[:, cc * P : (cc + 1) * P],
                                identity,
                                is_transpose=True,
                                start=(q == 0),
                                stop=(q == 3),
                            )
                        nc.any.tensor_copy(
                            xtb[:, g * 4 : (g + 1) * 4,
                                sub * P : (sub + 1) * P],
                            pt,
                        )
                nc.gpsimd.dma_start(out=XTd[tt][:], in_=xtb)

            def load_xt(tt, name):
                xt = blk_tile(name)
                nc.sync.dma_start(out=xt, in_=XTd[tt][:])
                return xt

            # phase 1: Q^T (query rows = blocks 0..1) -> QTd
            for tt in range(TQ // XBLK):
                xt = load_xt(tt, f"xt_q{tt}")
                for hc in range(HC):
                    ps = ppsum.tile([P, XBLK], f32, tag="pp")
                    for cc in range(CC):
                        nc.tensor.matmul(
                            ps,
                            w_q[:, cc, hc * P : (hc + 1) * P],
                            xt[:, cc, :],
                            start=(cc == 0),
                            stop=(cc == CC - 1),
                        )
                    st = stage.tile([P, XBLK], mmdt, tag="st")
                    nc.any.tensor_copy(st, ps)
                    nc.gpsimd.dma_start(
                        out=QTd[hc, :, tt * XBLK : (tt + 1) * XBLK], in_=st
                    )

            # phase 2: K^T -> KTd
            w_k = load_w(wk, "wk_sb")
            for tt in range(NTT):
                xt = load_xt(tt, f"xt_k{tt}")
                for hc in range(HC):
                    ps = ppsum.tile([P, XBLK], f32, tag="pp")
                    for cc in range(CC):
                        nc.tensor.matmul(
                            ps,
                            w_k[:, cc, hc * P : (hc + 1) * P],
                            xt[:, cc, :],
                            start=(cc == 0),
                            stop=(cc == CC - 1),
                        )
                    st = stage.tile([P, XBLK], mmdt, tag="st")
                    nc.any.tensor_copy(st, ps)
                    nc.gpsimd.dma_start(
                        out=KTd[hc, :, tt * XBLK : (tt + 1) * XBLK], in_=st
                    )

            # phase 3: V -> SBUF resident
            w_v = load_w(wv, "wv_sb")
            for tt in range(NTT):
                xt = load_xt(tt, f"xt_v{tt}")
                for sub in range(XBLK // P):
                    tkc = tt * (XBLK // P) + sub
                    for ht in range(H // 512):
                        ps = ppsum.tile([P, 512], f32, tag="pp")
                        for cc in range(CC):
                            nc.tensor.matmul(
                                ps,
                                xt[:, cc, sub * P : (sub + 1) * P],
                                w_v[:, cc, ht * 512 : (ht + 1) * 512],
                                start=(cc == 0),
                                stop=(cc == CC - 1),
                            )
                        nc.any.tensor_copy(
                            V[:, tkc, ht * 512 : (ht + 1) * 512], ps
                        )

        # ---------------- attention phase ----------------
        with ExitStack() as actx:
            et_pool = actx.enter_context(tc.tile_pool(name="et", bufs=2))
            spsum = actx.enter_context(
                tc.tile_pool(name="spsum", bufs=2, space="PSUM")
            )
            opsum = actx.enter_context(
                tc.tile_pool(name="opsum", bufs=4, space="PSUM")
            )
            dpsum = actx.enter_context(
                tc.tile_pool(name="dpsum", bufs=2, space="PSUM")
            )
            small = actx.enter_context(tc.tile_pool(name="small", bufs=4))
            ostage = actx.enter_context(tc.tile_pool(name="ostage", bufs=4))

            for j in range(NJ):
                q0 = j * TQT
                qt = blk_tile(f"qt_{j}")
                nc.sync.dma_start(
                    out=qt,
                    in_=QTd[:, :, q0 : q0 + TQT].rearrange("hc p t -> p hc t"),
                )
                et = et_pool.tile([P, NTK, TQT], mmdt, tag="et")
                actives = _active_tkcs(j)
                tts = sorted(set(tkc // (XBLK // P) for tkc in actives))
                kts = {}
                for tt in tts:
                    kt = blk_tile(f"kt_{j}_{tt}")
                    nc.sync.dma_start(
                        out=kt,
                        in_=KTd[:, :, tt * XBLK : (tt + 1) * XBLK].rearrange(
                            "hc p t -> p hc t"
                        ),
                    )
                    kts[tt] = kt
                # all 4 per-qc denominators share one PSUM bank
                d_ps = dpsum.tile([P, NQC, 2], f32, tag="dp")

                # scores + exp (+ mask) for every active key chunk
                for tkc in actives:
                    kt = kts[tkc // (XBLK // P)]
                    k0 = (tkc % (XBLK // P)) * P
                    sp = spsum.tile([P, TQT], f32, tag="sp", name=f"sp_{tkc}")
                    for hc in range(HC):
                        nc.tensor.matmul(
                            sp,
                            kt[:, hc, k0 : k0 + P],
                            qt[:, hc, :],
                            start=(hc == 0),
                            stop=(hc == HC - 1),
                        )
                    bias = tail_sb if tkc >= NTK // 2 else 0.0
                    nc.scalar.activation(
                        et[:, tkc, :],
                        sp,
                        mybir.ActivationFunctionType.Exp,
                        bias=bias,
                        scale=SCALE,
                    )
                    # diagonal-crossing tiles: triangular mask in rolled coords
                    if TQT * j <= tkc * P < TQT * (j + 1):
                        nc.gpsimd.affine_select(
                            out=et[:, tkc, :],
                            in_=et[:, tkc, :],
                            compare_op=mybir.AluOpType.is_ge,
                            fill=0.0,
                            base=TQT * j - P * tkc,
                            pattern=[[1, TQT]],
                            channel_multiplier=-1,
                        )
                    # denominator accumulation (one bank, single group)
                    for qc in range(NQC):
                        nc.tensor.matmul(
                            d_ps[:, qc, :],
                            et[:, tkc, qc * P : (qc + 1) * P],
                            ones,
                            start=(tkc == actives[0] and qc == 0),
                            stop=(tkc == actives[-1] and qc == NQC - 1),
                        )

                # O accumulation: one ht at a time so PSUM fits (4 qc tiles)
                for ht in range(2):
                    o_ps = [
                        opsum.tile([P, 512], f32, tag="op", name=f"o_{qc}")
                        for qc in range(NQC)
                    ]
                    for i, tkc in enumerate(actives):
                        for qc in range(NQC):
                            nc.tensor.matmul(
                                o_ps[qc],
                                et[:, tkc, qc * P : (qc + 1) * P],
                                V[:, tkc, ht * 512 : (ht + 1) * 512],
                                start=(i == 0),
                                stop=(i == len(actives) - 1),
                            )
                    for qc in range(NQC):
                        rec = small.tile([P, 1], f32, tag="rec")
                        nc.vector.reciprocal(rec, d_ps[:, qc, 0:1])
                        ot = ostage.tile([P, 512], f32, tag="ot")
                        nc.vector.tensor_scalar_mul(ot, o_ps[qc], rec)
                        nc.gpsimd.dma_start(
                            out=out_ap[
                                q0 + qc * P : q0 + (qc + 1) * P,
                                ht * 512 : (ht + 1) * 512,
                            ],
                            in_=ot,
                        )


def build_nc():
    import concourse.mybir as mybir
    import concourse.tile as tile
    from concourse import bacc

    nc = bacc.Bacc(
        "TRN2",
        target_bir_lowering=False,
        debug=False,
        num_devices=NCORES,
    )
    f32 = mybir.dt.float32
    mmdt = getattr(mybir.dt, MM_DTYPE)
    xr = nc.dram_tensor("xr", [T, C], f32, kind="ExternalInput").ap()
    wq = nc.dram_tensor("wq", [C, H], mmdt, kind="ExternalInput").ap()
    wk = nc.dram_tensor("wk", [C, H], mmdt, kind="ExternalInput").ap()
    wv = nc.dram_tensor("wv", [C, H], mmdt, kind="ExternalInput").ap()
    tailbias = nc.dram_tensor(
        "tailbias", [P, 1], f32, kind="ExternalInput"
    ).ap()
    out = nc.dram_tensor("out", [TQ, H], f32, kind="ExternalOutput").ap()
    with tile.TileContext(nc) as tc:
        _attn_body(tc, out, xr, wq, wk, wv, tailbias)
    nc.compile()
    return nc


def make_in_maps(x, Wk, Wq, Wv):
    x = np.asarray(x, dtype=np.float32)
    Wk = np.ascontiguousarray(np.asarray(Wk, dtype=np.float32))
    Wq = np.ascontiguousarray(np.asarray(Wq, dtype=np.float32))
    Wv = np.ascontiguousarray(np.asarray(Wv, dtype=np.float32))
    in_maps = []
    for c in range(NCORES):
        b, qh = divmod(c, 2)
        xb = x[b]
        if qh == 0:
            xr = np.ascontiguousarray(xb)
            tail = np.full((P, 1), TAIL_BIAS, dtype=np.float32)
        else:
            xr = np.ascontiguousarray(
                np.concatenate([xb[TQ:], xb[:TQ]], axis=0)
            )
            tail = np.zeros((P, 1), dtype=np.float32)
        in_maps.append(
            {"xr": xr, "wq": Wq, "wk": Wk, "wv": Wv, "tailbias": tail}
        )
    return in_maps


def assemble_out(results):
    out = np.empty((B, T, H), dtype=np.float32)
    for c in range(NCORES):
        b, qh = divmod(c, 2)
        out[b, qh * TQ : (qh + 1) * TQ] = results[c]["out"]
    return out


def kernel(x, Wk, Wq, Wv):
    from concourse import bass_utils

    nc = build_nc()
    in_maps = make_in_maps(x, Wk, Wq, Wv)
    res = bass_utils.run_bass_kernel_spmd(
        nc, in_maps, core_ids=list(range(NCORES))
    )
    return assemble_out(res.results)


# revision 17
# speedup vs baseline: 1.2742x; 1.0747x over previous
"""Causal single-head attention (B=4, T=2048, C=H=1024) on 8 TRN2 NeuronCores.

Sharding: core = (batch b, query half qh).  Each core computes attention for
1024 queries of one batch against all 2048 keys of that batch.  The host
passes x ROLLED so the core's own query rows are always rows [0, 1024) of its
input.  In rolled coordinates the causal mask is:
  - keys [0, 1024)   (own half):  triangular mask f >= p  (core independent)
  - keys [1024, 2048) (other half): all-keep or all-drop depending on which
    half this core owns.  Implemented as a per-core bias input (0 or -30)
    added inside the exp activation: exp(s - 30) ~ 5e-13 ~ 0.
Softmax uses no max subtraction (logits are ~N(0, 0.33), |s| < ~2, so exp is
fp32-safe); the denominator is obtained by matmuls against a ones vector and
normalization is reciprocal+multiply.  Fully-masked score tiles are skipped.

Structure (all matmul moving operands 512 wide; f32r ~1cyc/row, LDWEIGHTS
hides under the 213ns stream):
  phase 0: PE-transpose x -> x^T, staged to DRAM in 512-col blocks (done
           ONCE; transposes don't count as PE activity for the HAM clock
           gate, so keeping them out of the matmul phases keeps 2.4GHz).
           4 transposes share one PSUM bank -> one 512-wide evacuation.
  phase 1-3: pure matmul streams: Q^T -> DRAM, K^T -> DRAM, V -> SBUF
  attention: stream Q^T/K^T from DRAM; scores S^T = K^T.T @ Q^T per key
           chunk; exp on ScalarE; triangular mask via affine_select; O and
           denominator accumulate in PSUM; normalize; DMA out.
All 16KB [P, 8, 512] staging tiles (x^T blocks, q^T, k^T spans) share ONE
pool tag so there are no pool-scope barriers between phases and prefetches
cross phase boundaries.  Loads go on Sync (HWDGE), stores on GpSimd (SWDGE).
Weight tiles double-buffer from kernel start.
"""

import math
import sys

sys.path.insert(0, "/opt/trn_rl_repo")

import numpy as np

B, T, C, H = 4, 2048, 1024, 1024
NCORES = 8
TQ = T // 2          # queries per core
P = 128              # partitions
CC = C // P          # contraction chunks for projections
HC = H // P          # contraction chunks for scores
NTK = T // P         # key chunks (16)
TQT = 512            # tq tile width in attention phase
NJ = TQ // TQT       # 2 tq tiles
NQC = TQT // P       # 4 query chunks of 128 per tq tile
XBLK = 512           # x^T columns per block
NTT = T // XBLK      # 4 blocks
SCALE = 1.0 / math.sqrt(H)
TAIL_BIAS = -30.0

MM_DTYPE = "float32r"


def _active_tkcs(j):
    """Key chunks contributing to tq tile j (rolled coords)."""
    nblk = TQT // P
    return [
        tkc for tkc in range(NTK) if tkc >= NTK // 2 or tkc < nblk * (j + 1)
    ]


def _attn_body(tc, out_ap, xr, wq, wk, wv, tailbias):
    import concourse.mybir as mybir
    from concourse.masks import make_identity

    nc = tc.nc
    f32 = mybir.dt.float32
    mmdt = getattr(mybir.dt, MM_DTYPE)

    from contextlib import ExitStack

    with ExitStack() as ctx:
        consts = ctx.enter_context(tc.tile_pool(name="consts", bufs=1))
        ones_f32 = consts.tile([P, 2], f32)
        nc.vector.memset(ones_f32, 1.0)
        ones = consts.tile([P, 2], mmdt)
        nc.vector.tensor_copy(ones, ones_f32)
        tail_sb = consts.tile([P, 1], f32)
        nc.sync.dma_start(out=tail_sb, in_=tailbias)
        warm = consts.tile([P, 1], f32)
        nc.scalar.activation(warm, tail_sb, mybir.ActivationFunctionType.Exp)

        big = ctx.enter_context(tc.tile_pool(name="big", bufs=1))
        V = big.tile([P, NTK, H], mmdt)    # V: [tk, h], 64KB/part resident

        # [P, 8, 512] staging tiles: x^T blocks, q^T, k^T spans — one tag
        blk_pool = ctx.enter_context(tc.tile_pool(name="blk", bufs=3))

        def blk_tile(name):
            return blk_pool.tile([P, 8, XBLK], mmdt, tag="blk", name=name)

        dram = ctx.enter_context(
            tc.tile_pool(name="dram", bufs=1, space="DRAM")
        )
        # x^T: one tile per 512-col block, [p][cc][t] contiguous per partition
        XTd = [
            dram.tile([P, CC, XBLK], mmdt, name=f"xtd_{tt}")
            for tt in range(NTT)
        ]
        QTd = dram.tile([HC, P, TQ], mmdt)   # Q^T: [hc, h, tq]
        KTd = dram.tile([HC, P, T], mmdt)    # K^T: [hc, h, tk]

        # ------ phases 0-3: transpose + projections (shared W pool) --------
        with ExitStack() as pctx:
            wpool = pctx.enter_context(tc.tile_pool(name="wpool", bufs=2))
            idp = pctx.enter_context(tc.tile_pool(name="idp", bufs=1))
            xrow_pool = pctx.enter_context(tc.tile_pool(name="xrow", bufs=2))
            tpsum = pctx.enter_context(
                tc.tile_pool(name="tpsum", bufs=2, space="PSUM")
            )
            ppsum = pctx.enter_context(
                tc.tile_pool(name="ppsum", bufs=6, space="PSUM")
            )
            stage = pctx.enter_context(tc.tile_pool(name="pstage", bufs=3))

            def load_w(w_dram, name):
                w_sb = wpool.tile([P, CC, H], mmdt, tag="w", name=name)
                wr = w_dram.rearrange("(cc p) h -> p cc h", p=P)
                nc.sync.dma_start(out=w_sb[:, :, 0:512], in_=wr[:, :, 0:512])
                nc.sync.dma_start(out=w_sb[:, :, 512:H], in_=wr[:, :, 512:H])
                return w_sb

            w_q = load_w(wq, "wq_sb")  # prefetch from kernel start
            w_k = load_w(wk, "wk_sb")
            identity = idp.tile([P, P], f32)
            make_identity(nc, identity)

            # transpose block tt -> xtb tile (+ store to DRAM for the V pass)
            def make_xtb(tt):
                xtb = blk_tile(f"xtb_{tt}")
                steps = []
                for sub in range(XBLK // P):
                    tch = tt * (XBLK // P) + sub
                    xrow = xrow_pool.tile(
                        [P, C], f32, tag="xr", name=f"xrow_{tch}"
                    )
                    nc.sync.dma_start(
                        out=xrow, in_=xr[tch * P : (tch + 1) * P, :]
                    )
                    for g in range(2):  # 4 transposes share one PSUM bank

                        def step(xrow=xrow, g=g, sub=sub):
                            pt = tpsum.tile([P, 4, P], f32, tag="tp")
                            for q in range(4):
                                cc = g * 4 + q
                                nc.tensor.matmul(
                                    pt[:, q, :],
                                    xrow[:, cc * P : (cc + 1) * P],
                                    identity,
                                    is_transpose=True,
                                    start=(q == 0),
                                    stop=(q == 3),
                                )
                            nc.any.tensor_copy(
                                xtb[:, g * 4 : (g + 1) * 4,
                                    sub * P : (sub + 1) * P],
                                pt,
                            )

                        steps.append(step)
                return xtb, steps

            def flush(steps):
                for s in steps:
                    s()

            # Q^T block from xtb (blocks 0..1) -> QTd
            def qt_mms(tt, xt, interleave=()):
                it = iter(interleave)
                for hc in range(HC):
                    ps = ppsum.tile([P, XBLK], f32, tag="pp")
                    for cc in range(CC):
                        nc.tensor.matmul(
                            ps,
                            w_q[:, cc, hc * P : (hc + 1) * P],
                            xt[:, cc, :],
                            start=(cc == 0),
                            stop=(cc == CC - 1),
                        )
                    st = stage.tile([P, XBLK], mmdt, tag="st")
                    nc.any.tensor_copy(st, ps)
                    nc.gpsimd.dma_start(
                        out=QTd[hc, :, tt * XBLK : (tt + 1) * XBLK], in_=st
                    )
                    step = next(it, None)
                    if step:
                        step()

            # K^T block from xtb -> KTd
            def kt_mms(tt, xt, interleave=()):
                it = iter(interleave)
                for hc in range(HC):
                    ps = ppsum.tile([P, XBLK], f32, tag="pp")
                    for cc in range(CC):
                        nc.tensor.matmul(
                            ps,
                            w_k[:, cc, hc * P : (hc + 1) * P],
                            xt[:, cc, :],
                            start=(cc == 0),
                            stop=(cc == CC - 1),
                        )
                    st = stage.tile([P, XBLK], mmdt, tag="st")
                    nc.any.tensor_copy(st, ps)
                    nc.gpsimd.dma_start(
                        out=KTd[hc, :, tt * XBLK : (tt + 1) * XBLK], in_=st
                    )
                    step = next(it, None)
                    if step:
                        step()

            # Interleaved transpose + Q^T + K^T over the 4 blocks: the next
            # block's transposes are spliced between matmul groups so the HAM
            # clock gate never sees a long transpose-only window.
            xtb0, steps0 = make_xtb(0)
            flush(steps0)
            nc.gpsimd.dma_start(out=XTd[0][:], in_=xtb0)
            xtbs = {0: xtb0}
            for tt in range(NTT):
                nxt = tt + 1
                if nxt < NTT:
                    xtb_n, steps_n = make_xtb(nxt)
                else:
                    xtb_n, steps_n = None, []
                it = iter(steps_n)
                if tt < TQ // XBLK:
                    qt_mms(tt, xtbs[tt], it)
                kt_mms(tt, xtbs[tt], it)
                flush(list(it))
                if xtb_n is not None:
                    nc.gpsimd.dma_start(out=XTd[nxt][:], in_=xtb_n)
                    xtbs[nxt] = xtb_n
                del xtbs[tt]

            def load_xt(tt, name):
                xt = blk_tile(name)
                nc.sync.dma_start(out=xt, in_=XTd[tt][:])
                return xt

            # V pass: x^T streamed back from DRAM -> V resident
            w_v = load_w(wv, "wv_sb")
            for tt in range(NTT):
                xt = load_xt(tt, f"xt_v{tt}")
                for sub in range(XBLK // P):
                    tkc = tt * (XBLK // P) + sub
                    for ht in range(H // 512):
                        ps = ppsum.tile([P, 512], f32, tag="pp")
                        for cc in range(CC):
                            nc.tensor.matmul(
                                ps,
                                xt[:, cc, sub * P : (sub + 1) * P],
                                w_v[:, cc, ht * 512 : (ht + 1) * 512],
                                start=(cc == 0),
                                stop=(cc == CC - 1),
                            )
                        nc.any.tensor_copy(
                            V[:, tkc, ht * 512 : (ht + 1) * 512], ps
                        )

        # ---------------- attention phase ----------------
        with ExitStack() as actx:
            et_pool = actx.enter_context(tc.tile_pool(name="et", bufs=2))
            spsum = actx.enter_context(
                tc.tile_pool(name="spsum", bufs=2, space="PSUM")
            )
            opsum = actx.enter_context(
                tc.tile_pool(name="opsum", bufs=4, space="PSUM")
            )
            dpsum = actx.enter_context(
                tc.tile_pool(name="dpsum", bufs=2, space="PSUM")
            )
            small = actx.enter_context(tc.tile_pool(name="small", bufs=4))
            ostage = actx.enter_context(tc.tile_pool(name="ostage", bufs=4))

            for j in range(NJ):
                q0 = j * TQT
                qt = blk_tile(f"qt_{j}")
                nc.sync.dma_start(
                    out=qt,
                    in_=QTd[:, :, q0 : q0 + TQT].rearrange("hc p t -> p hc t"),
                )
                et = et_pool.tile([P, NTK, TQT], mmdt, tag="et")
                actives = _active_tkcs(j)
                tts = sorted(set(tkc // (XBLK // P) for tkc in actives))
                kts = {}
                for tt in tts:
                    kt = blk_tile(f"kt_{j}_{tt}")
                    nc.sync.dma_start(
                        out=kt,
                        in_=KTd[:, :, tt * XBLK : (tt + 1) * XBLK].rearrange(
                            "hc p t -> p hc t"
                        ),
                    )
                    kts[tt] = kt
                # all 4 per-qc denominators share one PSUM bank
                d_ps = dpsum.tile([P, NQC, 2], f32, tag="dp")

                # scores + exp (+ mask) for every active key chunk
                for tkc in actives:
                    kt = kts[tkc // (XBLK // P)]
                    k0 = (tkc % (XBLK // P)) * P
                    sp = spsum.tile([P, TQT], f32, tag="sp", name=f"sp_{tkc}")
                    for hc in range(HC):
                        nc.tensor.matmul(
                            sp,
                            kt[:, hc, k0 : k0 + P],
                            qt[:, hc, :],
                            start=(hc == 0),
                            stop=(hc == HC - 1),
                        )
                    bias = tail_sb if tkc >= NTK // 2 else 0.0
                    nc.scalar.activation(
                        et[:, tkc, :],
                        sp,
                        mybir.ActivationFunctionType.Exp,
                        bias=bias,
                        scale=SCALE,
                    )
                    # diagonal-crossing tiles: triangular mask in rolled coords
                    if TQT * j <= tkc * P < TQT * (j + 1):
                        nc.gpsimd.affine_select(
                            out=et[:, tkc, :],
                            in_=et[:, tkc, :],
                            compare_op=mybir.AluOpType.is_ge,
                            fill=0.0,
                            base=TQT * j - P * tkc,
                            pattern=[[1, TQT]],
                            channel_multiplier=-1,
                        )
                    # denominator accumulation (one bank, single group)
                    for qc in range(NQC):
                        nc.tensor.matmul(
                            d_ps[:, qc, :],
                            et[:, tkc, qc * P : (qc + 1) * P],
                            ones,
                            start=(tkc == actives[0] and qc == 0),
                            stop=(tkc == actives[-1] and qc == NQC - 1),
                        )

                # O accumulation: one ht at a time so PSUM fits (4 qc tiles)
                for ht in range(2):
                    o_ps = [
                        opsum.tile([P, 512], f32, tag="op", name=f"o_{qc}")
                        for qc in range(NQC)
                    ]
                    for i, tkc in enumerate(actives):
                        for qc in range(NQC):
                            nc.tensor.matmul(
                                o_ps[qc],
                                et[:, tkc, qc * P : (qc + 1) * P],
                                V[:, tkc, ht * 512 : (ht + 1) * 512],
                                start=(i == 0),
                                stop=(i == len(actives) - 1),
                            )
                    for qc in range(NQC):
                        rec = small.tile([P, 1], f32, tag="rec")
                        nc.vector.reciprocal(rec, d_ps[:, qc, 0:1])
                        ot = ostage.tile([P, 512], f32, tag="ot")
                        nc.vector.tensor_scalar_mul(ot, o_ps[qc], rec)
                        nc.gpsimd.dma_start(
                            out=out_ap[
                                q0 + qc * P : q0 + (qc + 1) * P,
                                ht * 512 : (ht + 1) * 512,
                            ],
                            in_=ot,
                        )


def build_nc():
    import concourse.mybir as mybir
    import concourse.tile as tile
    from concourse import bacc

    nc = bacc.Bacc(
        "TRN2",
        target_bir_lowering=False,
        debug=False,
        num_devices=NCORES,
    )
    f32 = mybir.dt.float32
    mmdt = getattr(mybir.dt, MM_DTYPE)
    xr = nc.dram_tensor("xr", [T, C], f32, kind="ExternalInput").ap()
    wq = nc.dram_tensor("wq", [C, H], mmdt, kind="ExternalInput").ap()
    wk = nc.dram_tensor("wk", [C, H], mmdt, kind="ExternalInput").ap()
    wv = nc.dram_tensor("wv", [C, H], mmdt, kind="ExternalInput").ap()
    tailbias = nc.dram_tensor(
        "tailbias", [P, 1], f32, kind="ExternalInput"
    ).ap()
    out = nc.dram_tensor("out", [TQ, H], f32, kind="ExternalOutput").ap()
    with tile.TileContext(nc) as tc:
        _attn_body(tc, out, xr, wq, wk, wv, tailbias)
    nc.compile()
    return nc


def make_in_maps(x, Wk, Wq, Wv):
    x = np.asarray(x, dtype=np.float32)
    Wk = np.ascontiguousarray(np.asarray(Wk, dtype=np.float32))
    Wq = np.ascontiguousarray(np.asarray(Wq, dtype=np.float32))
    Wv = np.ascontiguousarray(np.asarray(Wv, dtype=np.float32))
    in_maps = []
    for c in range(NCORES):
        b, qh = divmod(c, 2)
        xb = x[b]
        if qh == 0:
            xr = np.ascontiguousarray(xb)
            tail = np.full((P, 1), TAIL_BIAS, dtype=np.float32)
        else:
            xr = np.ascontiguousarray(
                np.concatenate([xb[TQ:], xb[:TQ]], axis=0)
            )
            tail = np.zeros((P, 1), dtype=np.float32)
        in_maps.append(
            {"xr": xr, "wq": Wq, "wk": Wk, "wv": Wv, "tailbias": tail}
        )
    return in_maps


def assemble_out(results):
    out = np.empty((B, T, H), dtype=np.float32)
    for c in range(NCORES):
        b, qh = divmod(c, 2)
        out[b, qh * TQ : (qh + 1) * TQ] = results[c]["out"]
    return out


def kernel(x, Wk, Wq, Wv):
    from concourse import bass_utils

    nc = build_nc()
    in_maps = make_in_maps(x, Wk, Wq, Wv)
    res = bass_utils.run_bass_kernel_spmd(
        nc, in_maps, core_ids=list(range(NCORES))
    )
    return assemble_out(res.results)


# revision 25
# speedup vs baseline: 1.4080x; 1.1050x over previous
"""Causal single-head attention (B=4, T=2048, C=H=1024) on 8 TRN2 NeuronCores.

Sharding: core = (batch b, query half qh).  Each core computes attention for
1024 queries of one batch against all 2048 keys of that batch.  The host
passes x ROLLED so the core's own query rows are always rows [0, 1024) of its
input.  In rolled coordinates the causal mask is:
  - keys [0, 1024)   (own half):  triangular mask f >= p  (core independent)
  - keys [1024, 2048) (other half): all-keep or all-drop depending on which
    half this core owns.  Implemented as a per-core bias input (0 or -30)
    added inside the exp activation: exp(s - 30) ~ 5e-13 ~ 0.
Softmax uses no max subtraction (logits are ~N(0, 0.33), |s| < ~2, so exp is
fp32-safe); the denominator is obtained by matmuls against a ones vector and
normalization is reciprocal+multiply.  Fully-masked score tiles are skipped.

Structure (all matmul moving operands 512 wide; f32r ~1cyc/row, LDWEIGHTS
hides under the 213ns stream):
  phase 0: PE-transpose x -> x^T, staged to DRAM in 512-col blocks (done
           ONCE; transposes don't count as PE activity for the HAM clock
           gate, so keeping them out of the matmul phases keeps 2.4GHz).
           4 transposes share one PSUM bank -> one 512-wide evacuation.
  phase 1-3: pure matmul streams: Q^T -> DRAM, K^T -> DRAM, V -> SBUF
  attention: stream Q^T/K^T from DRAM; scores S^T = K^T.T @ Q^T per key
           chunk; exp on ScalarE; triangular mask via affine_select; O and
           denominator accumulate in PSUM; normalize; DMA out.
All 16KB [P, 8, 512] staging tiles (x^T blocks, q^T, k^T spans) share ONE
pool tag so there are no pool-scope barriers between phases and prefetches
cross phase boundaries.  Loads go on Sync (HWDGE), stores on GpSimd (SWDGE).
Weight tiles double-buffer from kernel start.
"""

import math
import sys

sys.path.insert(0, "/opt/trn_rl_repo")

import numpy as np

B, T, C, H = 4, 2048, 1024, 1024
NCORES = 8
TQ = T // 2          # queries per core
P = 128              # partitions
CC = C // P          # contraction chunks for projections
HC = H // P          # contraction chunks for scores
NTK = T // P         # key chunks (16)
TQT = 512            # tq tile width in attention phase
NJ = TQ // TQT       # 2 tq tiles
NQC = TQT // P       # 4 query chunks of 128 per tq tile
XBLK = 512           # x^T columns per block
NTT = T // XBLK      # 4 blocks
SCALE = 1.0 / math.sqrt(H)
TAIL_BIAS = -30.0

MM_DTYPE = "float32r"


def _active_tkcs(j):
    """Key chunks contributing to tq tile j (rolled coords)."""
    nblk = TQT // P
    return [
        tkc for tkc in range(NTK) if tkc >= NTK // 2 or tkc < nblk * (j + 1)
    ]


def _attn_body(tc, out_ap, xr, wq, wk, wv, tailbias):
    import concourse.mybir as mybir
    from concourse.masks import make_identity

    nc = tc.nc
    f32 = mybir.dt.float32
    mmdt = getattr(mybir.dt, MM_DTYPE)

    from contextlib import ExitStack

    with ExitStack() as ctx:
        consts = ctx.enter_context(tc.tile_pool(name="consts", bufs=1))
        ones_f32 = consts.tile([P, 2], f32)
        nc.vector.memset(ones_f32, 1.0)
        ones = consts.tile([P, 2], mmdt)
        nc.vector.tensor_copy(ones, ones_f32)
        tail_sb = consts.tile([P, 1], f32)
        nc.sync.dma_start(out=tail_sb, in_=tailbias)
        warm = consts.tile([P, 1], f32)
        nc.scalar.activation(warm, tail_sb, mybir.ActivationFunctionType.Exp)
        identity = consts.tile([P, P], f32)
        make_identity(nc, identity)

        big = ctx.enter_context(tc.tile_pool(name="big", bufs=1))
        V = big.tile([P, NTK, H], mmdt)    # V: [tk, h], 64KB/part resident

        # [P, 8, 512] staging tiles: x^T blocks, q^T, k^T spans — one tag
        blk_pool = ctx.enter_context(tc.tile_pool(name="blk", bufs=3))

        def blk_tile(name):
            return blk_pool.tile([P, 8, XBLK], mmdt, tag="blk", name=name)

        dram = ctx.enter_context(
            tc.tile_pool(name="dram", bufs=1, space="DRAM")
        )
        # x^T: one tile per 512-col block, [p][cc][t] contiguous per partition
        XTd = [
            dram.tile([P, CC, XBLK], mmdt, name=f"xtd_{tt}")
            for tt in range(NTT)
        ]
        QTd = dram.tile([HC, P, TQ], mmdt)   # Q^T: [hc, h, tq]
        KTd = dram.tile([HC, P, T], mmdt)    # K^T: [hc, h, tk]

        # ------ phases 0-3: transpose + projections (shared W pool) --------
        with ExitStack() as pctx:
            wpool = pctx.enter_context(tc.tile_pool(name="wpool", bufs=2))
            xrow_pool = pctx.enter_context(tc.tile_pool(name="xrow", bufs=2))
            tpsum = pctx.enter_context(
                tc.tile_pool(name="tpsum", bufs=2, space="PSUM")
            )
            ppsum = pctx.enter_context(
                tc.tile_pool(name="ppsum", bufs=6, space="PSUM")
            )
            stage = pctx.enter_context(tc.tile_pool(name="pstage", bufs=3))

            def load_w(w_dram, name):
                # Scalar HWDGE queue: W descriptor generation (~5us/half)
                # must not head-of-line-block the xrow loads on Sync.
                w_sb = wpool.tile([P, CC, H], mmdt, tag="w", name=name)
                wr = w_dram.rearrange("(cc p) h -> p cc h", p=P)
                nc.scalar.dma_start(out=w_sb[:, :, 0:512], in_=wr[:, :, 0:512])
                nc.scalar.dma_start(out=w_sb[:, :, 512:H], in_=wr[:, :, 512:H])
                return w_sb

            # transpose block tt -> xtb tile (+ store to DRAM for the V pass)
            def make_xtb(tt):
                xtb = blk_tile(f"xtb_{tt}")
                steps = []
                for sub in range(XBLK // P):
                    tch = tt * (XBLK // P) + sub
                    xrow = xrow_pool.tile(
                        [P, C], f32, tag="xr", name=f"xrow_{tch}"
                    )
                    nc.sync.dma_start(
                        out=xrow, in_=xr[tch * P : (tch + 1) * P, :]
                    )
                    for g in range(2):  # 4 transposes share one PSUM bank

                        def step(xrow=xrow, g=g, sub=sub):
                            pt = tpsum.tile([P, 4, P], f32, tag="tp")
                            for q in range(4):
                                cc = g * 4 + q
                                nc.tensor.matmul(
                                    pt[:, q, :],
                                    xrow[:, cc * P : (cc + 1) * P],
                                    identity,
                                    is_transpose=True,
                                    start=(q == 0),
                                    stop=(q == 3),
                                )
                            nc.any.tensor_copy(
                                xtb[:, g * 4 : (g + 1) * 4,
                                    sub * P : (sub + 1) * P],
                                pt,
                            )

                        steps.append(step)
                return xtb, steps

            def flush(steps):
                for s in steps:
                    s()

            # Q^T block from xtb (blocks 0..1) -> QTd
            def qt_mms(tt, xt, interleave=()):
                it = iter(interleave)
                for hc in range(HC):
                    ps = ppsum.tile([P, XBLK], f32, tag="pp")
                    for cc in range(CC):
                        nc.tensor.matmul(
                            ps,
                            w_q[:, cc, hc * P : (hc + 1) * P],
                            xt[:, cc, :],
                            start=(cc == 0),
                            stop=(cc == CC - 1),
                        )
                    st = stage.tile([P, XBLK], mmdt, tag="st")
                    nc.any.tensor_copy(st, ps)
                    nc.gpsimd.dma_start(
                        out=QTd[hc, :, tt * XBLK : (tt + 1) * XBLK], in_=st
                    )
                    step = next(it, None)
                    if step:
                        step()

            # K^T block from xtb -> KTd
            def kt_mms(tt, xt, interleave=()):
                it = iter(interleave)
                for hc in range(HC):
                    ps = ppsum.tile([P, XBLK], f32, tag="pp")
                    for cc in range(CC):
                        nc.tensor.matmul(
                            ps,
                            w_k[:, cc, hc * P : (hc + 1) * P],
                            xt[:, cc, :],
                            start=(cc == 0),
                            stop=(cc == CC - 1),
                        )
                    st = stage.tile([P, XBLK], mmdt, tag="st")
                    nc.any.tensor_copy(st, ps)
                    nc.gpsimd.dma_start(
                        out=KTd[hc, :, tt * XBLK : (tt + 1) * XBLK], in_=st
                    )
                    step = next(it, None)
                    if step:
                        step()

            # Interleaved transpose + Q^T + K^T over the 4 blocks: the next
            # block's transposes are spliced between matmul groups so the HAM
            # clock gate never sees a long transpose-only window.  Block 0's
            # xrow loads are emitted before the W loads so they go first.
            xtb0, steps0 = make_xtb(0)
            w_q = load_w(wq, "wq_sb")
            w_k = load_w(wk, "wk_sb")
            flush(steps0)
            nc.gpsimd.dma_start(out=XTd[0][:], in_=xtb0)
            xtbs = {0: xtb0}
            for tt in range(NTT):
                nxt = tt + 1
                if nxt < NTT:
                    xtb_n, steps_n = make_xtb(nxt)
                else:
                    xtb_n, steps_n = None, []
                it = iter(steps_n)
                if tt < TQ // XBLK:
                    qt_mms(tt, xtbs[tt], it)
                kt_mms(tt, xtbs[tt], it)
                flush(list(it))
                if xtb_n is not None:
                    nc.gpsimd.dma_start(out=XTd[nxt][:], in_=xtb_n)
                    xtbs[nxt] = xtb_n
                del xtbs[tt]

            def load_xt(tt, name):
                xt = blk_tile(name)
                nc.sync.dma_start(out=xt, in_=XTd[tt][:])
                return xt

            # V pass: x^T streamed back from DRAM -> V resident
            w_v = load_w(wv, "wv_sb")
            for tt in range(NTT):
                xt = load_xt(tt, f"xt_v{tt}")
                for sub in range(XBLK // P):
                    tkc = tt * (XBLK // P) + sub
                    for ht in range(H // 512):
                        ps = ppsum.tile([P, 512], f32, tag="pp")
                        for cc in range(CC):
                            nc.tensor.matmul(
                                ps,
                                xt[:, cc, sub * P : (sub + 1) * P],
                                w_v[:, cc, ht * 512 : (ht + 1) * 512],
                                start=(cc == 0),
                                stop=(cc == CC - 1),
                            )
                        nc.any.tensor_copy(
                            V[:, tkc, ht * 512 : (ht + 1) * 512], ps
                        )

        # ---------------- attention phase ----------------
        with ExitStack() as actx:
            et_pool = actx.enter_context(tc.tile_pool(name="et", bufs=2))
            spsum = actx.enter_context(
                tc.tile_pool(name="spsum", bufs=2, space="PSUM")
            )
            opsum = actx.enter_context(
                tc.tile_pool(name="opsum", bufs=4, space="PSUM")
            )
            dpsum = actx.enter_context(
                tc.tile_pool(name="dpsum", bufs=1, space="PSUM")
            )
            dtpsum = actx.enter_context(
                tc.tile_pool(name="dtpsum", bufs=1, space="PSUM")
            )
            small = actx.enter_context(tc.tile_pool(name="small", bufs=4))
            ostage = actx.enter_context(tc.tile_pool(name="ostage", bufs=4))

            for j in range(NJ):
                q0 = j * TQT
                qt = blk_tile(f"qt_{j}")
                nc.sync.dma_start(
                    out=qt,
                    in_=QTd[:, :, q0 : q0 + TQT].rearrange("hc p t -> p hc t"),
                )
                et = et_pool.tile([P, NTK, TQT], mmdt, tag="et")
                actives = _active_tkcs(j)
                tts = sorted(set(tkc // (XBLK // P) for tkc in actives))
                kts = {}
                for tt in tts:
                    kt = blk_tile(f"kt_{j}_{tt}")
                    nc.sync.dma_start(
                        out=kt,
                        in_=KTd[:, :, tt * XBLK : (tt + 1) * XBLK].rearrange(
                            "hc p t -> p hc t"
                        ),
                    )
                    kts[tt] = kt
                # denominators: [2, tq] row-sum accumulator (ones stationary)
                d_ps = dpsum.tile([2, TQT], f32, tag="dp")

                # scores + exp (+ mask) for every active key chunk
                for tkc in actives:
                    kt = kts[tkc // (XBLK // P)]
                    k0 = (tkc % (XBLK // P)) * P
                    sp = spsum.tile([P, TQT], f32, tag="sp", name=f"sp_{tkc}")
                    for hc in range(HC):
                        nc.tensor.matmul(
                            sp,
                            kt[:, hc, k0 : k0 + P],
                            qt[:, hc, :],
                            start=(hc == 0),
                            stop=(hc == HC - 1),
                        )
                    bias = tail_sb if tkc >= NTK // 2 else 0.0
                    nc.scalar.activation(
                        et[:, tkc, :],
                        sp,
                        mybir.ActivationFunctionType.Exp,
                        bias=bias,
                        scale=SCALE,
                    )
                    # diagonal-crossing tiles: triangular mask in rolled coords
                    if TQT * j <= tkc * P < TQT * (j + 1):
                        nc.gpsimd.affine_select(
                            out=et[:, tkc, :],
                            in_=et[:, tkc, :],
                            compare_op=mybir.AluOpType.is_ge,
                            fill=0.0,
                            base=TQT * j - P * tkc,
                            pattern=[[1, TQT]],
                            channel_multiplier=-1,
                        )
                    # denominator accumulation: ones stationary, et moving
                    nc.tensor.matmul(
                        d_ps,
                        ones,
                        et[:, tkc, :],
                        start=(tkc == actives[0]),
                        stop=(tkc == actives[-1]),
                    )

                # transpose denominators into per-partition [tq, 1] layout
                dsb = small.tile([2, TQT], f32, tag="dsb")
                nc.vector.tensor_copy(dsb, d_ps)
                dt_ps = dtpsum.tile([P, NQC, 2], f32, tag="dt")
                for qc in range(NQC):
                    nc.tensor.matmul(
                        dt_ps[:, qc, :],
                        dsb[:, qc * P : (qc + 1) * P],
                        identity[0:2, 0:2],
                        is_transpose=True,
                        start=(qc == 0),
                        stop=(qc == NQC - 1),
                    )

                # O accumulation: one ht at a time so PSUM fits (4 qc tiles)
                for ht in range(2):
                    o_ps = [
                        opsum.tile([P, 512], f32, tag="op", name=f"o_{qc}")
                        for qc in range(NQC)
                    ]
                    for i, tkc in enumerate(actives):
                        for qc in range(NQC):
                            nc.tensor.matmul(
                                o_ps[qc],
                                et[:, tkc, qc * P : (qc + 1) * P],
                                V[:, tkc, ht * 512 : (ht + 1) * 512],
                                start=(i == 0),
                                stop=(i == len(actives) - 1),
                            )
                    for qc in range(NQC):
                        rec = small.tile([P, 1], f32, tag="rec")
                        nc.vector.reciprocal(rec, dt_ps[:, qc, 0:1])
                        ot = ostage.tile([P, 512], f32, tag="ot")
                        nc.vector.tensor_scalar_mul(ot, o_ps[qc], rec)
                        nc.gpsimd.dma_start(
                            out=out_ap[
                                q0 + qc * P : q0 + (qc + 1) * P,
                                ht * 512 : (ht + 1) * 512,
                            ],
                            in_=ot,
                        )


def build_nc():
    import concourse.mybir as mybir
    import concourse.tile as tile
    from concourse import bacc

    nc = bacc.Bacc(
        "TRN2",
        target_bir_lowering=False,
        debug=False,
        num_devices=NCORES,
    )
    f32 = mybir.dt.float32
    mmdt = getattr(mybir.dt, MM_DTYPE)
    xr = nc.dram_tensor("xr", [T, C], f32, kind="ExternalInput").ap()
    wq = nc.dram_tensor("wq", [C, H], mmdt, kind="ExternalInput").ap()
    wk = nc.dram_tensor("wk", [C, H], mmdt, kind="ExternalInput").ap()
    wv = nc.dram_tensor("wv", [C, H], mmdt, kind="ExternalInput").ap()
    tailbias = nc.dram_tensor(
        "tailbias", [P, 1], f32, kind="ExternalInput"
    ).ap()
    out = nc.dram_tensor("out", [TQ, H], f32, kind="ExternalOutput").ap()
    with tile.TileContext(nc) as tc:
        _attn_body(tc, out, xr, wq, wk, wv, tailbias)
    nc.compile()
    return nc


def make_in_maps(x, Wk, Wq, Wv):
    x = np.asarray(x, dtype=np.float32)
    Wk = np.ascontiguousarray(np.asarray(Wk, dtype=np.float32))
    Wq = np.ascontiguousarray(np.asarray(Wq, dtype=np.float32))
    Wv = np.ascontiguousarray(np.asarray(Wv, dtype=np.float32))
    in_maps = []
    for c in range(NCORES):
        b, qh = divmod(c, 2)
        xb = x[b]
        if qh == 0:
            xr = np.ascontiguousarray(xb)
            tail = np.full((P, 1), TAIL_BIAS, dtype=np.float32)
        else:
            xr = np.ascontiguousarray(
                np.concatenate([xb[TQ:], xb[:TQ]], axis=0)
            )
            tail = np.zeros((P, 1), dtype=np.float32)
        in_maps.append(
            {"xr": xr, "wq": Wq, "wk": Wk, "wv": Wv, "tailbias": tail}
        )
    return in_maps


def assemble_out(results):
    out = np.empty((B, T, H), dtype=np.float32)
    for c in range(NCORES):
        b, qh = divmod(c, 2)
        out[b, qh * TQ : (qh + 1) * TQ] = results[c]["out"]
    return out


def kernel(x, Wk, Wq, Wv):
    from concourse import bass_utils

    nc = build_nc()
    in_maps = make_in_maps(x, Wk, Wq, Wv)
    res = bass_utils.run_bass_kernel_spmd(
        nc, in_maps, core_ids=list(range(NCORES))
    )
    return assemble_out(res.results)


# revision 30
# speedup vs baseline: 1.4742x; 1.0470x over previous
"""Causal single-head attention (B=4, T=2048, C=H=1024) on 8 TRN2 NeuronCores.

Sharding: core = (batch b, query half qh).  Each core computes attention for
1024 queries of one batch against all 2048 keys of that batch.  The host
passes x ROLLED so the core's own query rows are always rows [0, 1024) of its
input.  In rolled coordinates the causal mask is:
  - keys [0, 1024)   (own half):  triangular mask f >= p  (core independent)
  - keys [1024, 2048) (other half): all-keep or all-drop depending on which
    half this core owns.  Implemented as a per-core bias input (0 or -30)
    added inside the exp activation: exp(s - 30) ~ 5e-13 ~ 0.
Softmax uses no max subtraction (logits are ~N(0, 0.33), |s| < ~2, so exp is
fp32-safe); the denominator is obtained by matmuls against a ones vector and
normalization is reciprocal+multiply.  Fully-masked score tiles are skipped.

Structure (all matmul moving operands 512 wide; f32r ~1cyc/row, LDWEIGHTS
hides under the 213ns stream):
  phase 0: PE-transpose x -> x^T, staged to DRAM in 512-col blocks (done
           ONCE; transposes don't count as PE activity for the HAM clock
           gate, so keeping them out of the matmul phases keeps 2.4GHz).
           4 transposes share one PSUM bank -> one 512-wide evacuation.
  phase 1-3: pure matmul streams: Q^T -> DRAM, K^T -> DRAM, V -> SBUF
  attention: stream Q^T/K^T from DRAM; scores S^T = K^T.T @ Q^T per key
           chunk; exp on ScalarE; triangular mask via affine_select; O and
           denominator accumulate in PSUM; normalize; DMA out.
All 16KB [P, 8, 512] staging tiles (x^T blocks, q^T, k^T spans) share ONE
pool tag so there are no pool-scope barriers between phases and prefetches
cross phase boundaries.  Loads go on Sync (HWDGE), stores on GpSimd (SWDGE).
Weight tiles double-buffer from kernel start.
"""

import math
import sys

sys.path.insert(0, "/opt/trn_rl_repo")

import numpy as np

B, T, C, H = 4, 2048, 1024, 1024
NCORES = 8
TQ = T // 2          # queries per core
P = 128              # partitions
CC = C // P          # contraction chunks for projections
HC = H // P          # contraction chunks for scores
NTK = T // P         # key chunks (16)
TQT = 512            # tq tile width in attention phase
NJ = TQ // TQT       # 2 tq tiles
NQC = TQT // P       # 4 query chunks of 128 per tq tile
XBLK = 512           # x^T columns per block
NTT = T // XBLK      # 4 blocks
SCALE = 1.0 / math.sqrt(H)
TAIL_BIAS = -30.0

MM_DTYPE = "float32r"


def _active_tkcs(j):
    """Key chunks contributing to tq tile j (rolled coords)."""
    nblk = TQT // P
    return [
        tkc for tkc in range(NTK) if tkc >= NTK // 2 or tkc < nblk * (j + 1)
    ]


def _attn_body(tc, out_ap, xr, wq, wk, wv, tailbias):
    import concourse.mybir as mybir
    from concourse.masks import make_identity

    nc = tc.nc
    f32 = mybir.dt.float32
    mmdt = getattr(mybir.dt, MM_DTYPE)

    from contextlib import ExitStack

    with ExitStack() as ctx:
        consts = ctx.enter_context(tc.tile_pool(name="consts", bufs=1))
        ones_f32 = consts.tile([P, 2], f32)
        nc.vector.memset(ones_f32, 1.0)
        ones = consts.tile([P, 2], mmdt)
        nc.vector.tensor_copy(ones, ones_f32)
        tail_sb = consts.tile([P, 1], f32)
        nc.sync.dma_start(out=tail_sb, in_=tailbias)
        identity = consts.tile([P, P], f32)
        make_identity(nc, identity)

        big = ctx.enter_context(tc.tile_pool(name="big", bufs=1))
        V = big.tile([P, NTK, H], mmdt)    # V: [tk, h], 64KB/part resident

        # [P, 8, 512] staging tiles: x^T blocks, q^T, k^T spans — one tag
        blk_pool = ctx.enter_context(tc.tile_pool(name="blk", bufs=3))

        def blk_tile(name):
            return blk_pool.tile([P, 8, XBLK], mmdt, tag="blk", name=name)

        dram = ctx.enter_context(
            tc.tile_pool(name="dram", bufs=1, space="DRAM")
        )
        # x^T: one tile per 512-col block, [p][cc][t] contiguous per partition
        XTd = [
            dram.tile([P, CC, XBLK], mmdt, name=f"xtd_{tt}")
            for tt in range(NTT)
        ]
        QTd = dram.tile([HC, P, TQ], mmdt)   # Q^T: [hc, h, tq]
        KTd = dram.tile([HC, P, T], mmdt)    # K^T: [hc, h, tk]

        # ------ phases 0-3: transpose + projections (shared W pool) --------
        with ExitStack() as pctx:
            wpool = pctx.enter_context(tc.tile_pool(name="wpool", bufs=2))
            xrow_pool = pctx.enter_context(tc.tile_pool(name="xrow", bufs=2))
            tpsum = pctx.enter_context(
                tc.tile_pool(name="tpsum", bufs=2, space="PSUM")
            )
            ppsum = pctx.enter_context(
                tc.tile_pool(name="ppsum", bufs=6, space="PSUM")
            )
            stage = pctx.enter_context(tc.tile_pool(name="pstage", bufs=3))

            def load_w(w_dram, name):
                # Scalar HWDGE queue: W descriptor generation (~5us/half)
                # must not head-of-line-block the xrow loads on Sync.
                w_sb = wpool.tile([P, CC, H], mmdt, tag="w", name=name)
                wr = w_dram.rearrange("(cc p) h -> p cc h", p=P)
                nc.scalar.dma_start(out=w_sb[:, :, 0:512], in_=wr[:, :, 0:512])
                nc.scalar.dma_start(out=w_sb[:, :, 512:H], in_=wr[:, :, 512:H])
                return w_sb

            # transpose block tt -> xtb tile (+ store to DRAM for the V pass)
            def make_xtb(tt):
                xtb = blk_tile(f"xtb_{tt}")
                steps = []
                for sub in range(XBLK // P):
                    tch = tt * (XBLK // P) + sub
                    xrow = xrow_pool.tile(
                        [P, C], f32, tag="xr", name=f"xrow_{tch}"
                    )
                    nc.sync.dma_start(
                        out=xrow, in_=xr[tch * P : (tch + 1) * P, :]
                    )
                    for g in range(2):  # 4 transposes share one PSUM bank

                        def step(xrow=xrow, g=g, sub=sub):
                            pt = tpsum.tile([P, 4, P], f32, tag="tp")
                            for q in range(4):
                                cc = g * 4 + q
                                nc.tensor.matmul(
                                    pt[:, q, :],
                                    xrow[:, cc * P : (cc + 1) * P],
                                    identity,
                                    is_transpose=True,
                                    start=(q == 0),
                                    stop=(q == 3),
                                )
                            nc.any.tensor_copy(
                                xtb[:, g * 4 : (g + 1) * 4,
                                    sub * P : (sub + 1) * P],
                                pt,
                            )

                        steps.append(step)
                return xtb, steps

            def flush(steps):
                for s in steps:
                    s()

            # Q^T block from xtb (blocks 0..1) -> QTd
            def qt_mms(tt, xt, interleave=()):
                it = iter(interleave)
                for hc in range(HC):
                    ps = ppsum.tile([P, XBLK], f32, tag="pp")
                    for cc in range(CC):
                        nc.tensor.matmul(
                            ps,
                            w_q[:, cc, hc * P : (hc + 1) * P],
                            xt[:, cc, :],
                            start=(cc == 0),
                            stop=(cc == CC - 1),
                        )
                    st = stage.tile([P, XBLK], mmdt, tag="st")
                    nc.any.tensor_copy(st, ps)
                    nc.gpsimd.dma_start(
                        out=QTd[hc, :, tt * XBLK : (tt + 1) * XBLK], in_=st
                    )
                    step = next(it, None)
                    if step:
                        step()

            # K^T block from xtb -> KTd
            def kt_mms(tt, xt, interleave=()):
                it = iter(interleave)
                for hc in range(HC):
                    ps = ppsum.tile([P, XBLK], f32, tag="pp")
                    for cc in range(CC):
                        nc.tensor.matmul(
                            ps,
                            w_k[:, cc, hc * P : (hc + 1) * P],
                            xt[:, cc, :],
                            start=(cc == 0),
                            stop=(cc == CC - 1),
                        )
                    st = stage.tile([P, XBLK], mmdt, tag="st")
                    nc.any.tensor_copy(st, ps)
                    nc.gpsimd.dma_start(
                        out=KTd[hc, :, tt * XBLK : (tt + 1) * XBLK], in_=st
                    )
                    step = next(it, None)
                    if step:
                        step()

            # Interleaved transpose + Q^T + K^T over the 4 blocks: the next
            # block's transposes are spliced between matmul groups so the HAM
            # clock gate never sees a long transpose-only window.  Block 0's
            # xrow loads are emitted before the W loads so they go first.
            xtb0, steps0 = make_xtb(0)
            w_q = load_w(wq, "wq_sb")
            w_k = load_w(wk, "wk_sb")
            # exp table prewarm; emitted after the W loads so the ~2.7us
            # ACT table fetch doesn't delay them on the Scalar queue
            warm = consts.tile([P, 1], f32)
            nc.scalar.activation(
                warm, tail_sb, mybir.ActivationFunctionType.Exp
            )
            flush(steps0)
            nc.gpsimd.dma_start(out=XTd[0][:], in_=xtb0)
            xtbs = {0: xtb0}
            for tt in range(NTT):
                nxt = tt + 1
                if nxt < NTT:
                    xtb_n, steps_n = make_xtb(nxt)
                else:
                    xtb_n, steps_n = None, []
                it = iter(steps_n)
                if tt < TQ // XBLK:
                    qt_mms(tt, xtbs[tt], it)
                kt_mms(tt, xtbs[tt], it)
                flush(list(it))
                if xtb_n is not None:
                    if nxt < 2:
                        # only blocks 0..1 need the DRAM staging roundtrip;
                        # blocks 2..3 stay live in SBUF for the V pass
                        nc.gpsimd.dma_start(out=XTd[nxt][:], in_=xtb_n)
                    xtbs[nxt] = xtb_n

            # V pass: blocks 3,2 straight from SBUF; 1,0 re-streamed from
            # DRAM (loads hidden under the 3,2 matmuls)
            w_v = load_w(wv, "wv_sb")
            v_srcs = [(3, xtbs[3]), (2, xtbs[2])]
            for tt in (1, 0):
                xt = blk_tile(f"xt_v{tt}")
                nc.sync.dma_start(out=xt, in_=XTd[tt][:])
                v_srcs.append((tt, xt))
            for tt, xt in v_srcs:
                for sub in range(XBLK // P):
                    tkc = tt * (XBLK // P) + sub
                    for ht in range(H // 512):
                        ps = ppsum.tile([P, 512], f32, tag="pp")
                        for cc in range(CC):
                            nc.tensor.matmul(
                                ps,
                                xt[:, cc, sub * P : (sub + 1) * P],
                                w_v[:, cc, ht * 512 : (ht + 1) * 512],
                                start=(cc == 0),
                                stop=(cc == CC - 1),
                            )
                        nc.any.tensor_copy(
                            V[:, tkc, ht * 512 : (ht + 1) * 512], ps
                        )

        # ---------------- attention phase ----------------
        with ExitStack() as actx:
            et_pool = actx.enter_context(tc.tile_pool(name="et", bufs=2))
            spsum = actx.enter_context(
                tc.tile_pool(name="spsum", bufs=2, space="PSUM")
            )
            opsum = actx.enter_context(
                tc.tile_pool(name="opsum", bufs=4, space="PSUM")
            )
            dpsum = actx.enter_context(
                tc.tile_pool(name="dpsum", bufs=1, space="PSUM")
            )
            dtpsum = actx.enter_context(
                tc.tile_pool(name="dtpsum", bufs=1, space="PSUM")
            )
            small = actx.enter_context(tc.tile_pool(name="small", bufs=4))
            ostage = actx.enter_context(tc.tile_pool(name="ostage", bufs=4))

            for j in range(NJ):
                q0 = j * TQT
                qt = blk_tile(f"qt_{j}")
                nc.sync.dma_start(
                    out=qt,
                    in_=QTd[:, :, q0 : q0 + TQT].rearrange("hc p t -> p hc t"),
                )
                et = et_pool.tile([P, NTK, TQT], mmdt, tag="et")
                # tail chunks first: their K^T/V tiles are produced earliest
                actives = sorted(_active_tkcs(j), key=lambda t: t < NTK // 2)
                tts = sorted(
                    set(tkc // (XBLK // P) for tkc in actives),
                    key=lambda t: t < NTT // 2,
                )
                kts = {}
                for tt in tts:
                    kt = blk_tile(f"kt_{j}_{tt}")
                    nc.sync.dma_start(
                        out=kt,
                        in_=KTd[:, :, tt * XBLK : (tt + 1) * XBLK].rearrange(
                            "hc p t -> p hc t"
                        ),
                    )
                    kts[tt] = kt
                # denominators: [2, tq] row-sum accumulator (ones stationary)
                d_ps = dpsum.tile([2, TQT], f32, tag="dp")

                # scores + exp (+ mask) for every active key chunk
                for tkc in actives:
                    kt = kts[tkc // (XBLK // P)]
                    k0 = (tkc % (XBLK // P)) * P
                    sp = spsum.tile([P, TQT], f32, tag="sp", name=f"sp_{tkc}")
                    for hc in range(HC):
                        nc.tensor.matmul(
                            sp,
                            kt[:, hc, k0 : k0 + P],
                            qt[:, hc, :],
                            start=(hc == 0),
                            stop=(hc == HC - 1),
                        )
                    bias = tail_sb if tkc >= NTK // 2 else 0.0
                    nc.scalar.activation(
                        et[:, tkc, :],
                        sp,
                        mybir.ActivationFunctionType.Exp,
                        bias=bias,
                        scale=SCALE,
                    )
                    # diagonal-crossing tiles: triangular mask in rolled coords
                    if TQT * j <= tkc * P < TQT * (j + 1):
                        nc.gpsimd.affine_select(
                            out=et[:, tkc, :],
                            in_=et[:, tkc, :],
                            compare_op=mybir.AluOpType.is_ge,
                            fill=0.0,
                            base=TQT * j - P * tkc,
                            pattern=[[1, TQT]],
                            channel_multiplier=-1,
                        )
                    # denominator accumulation: ones stationary, et moving
                    nc.tensor.matmul(
                        d_ps,
                        ones,
                        et[:, tkc, :],
                        start=(tkc == actives[0]),
                        stop=(tkc == actives[-1]),
                    )

                # transpose denominators into per-partition [tq, 1] layout
                dsb = small.tile([2, TQT], f32, tag="dsb")
                nc.vector.tensor_copy(dsb, d_ps)
                dt_ps = dtpsum.tile([P, NQC, 2], f32, tag="dt")
                for qc in range(NQC):
                    nc.tensor.matmul(
                        dt_ps[:, qc, :],
                        dsb[:, qc * P : (qc + 1) * P],
                        identity[0:2, 0:2],
                        is_transpose=True,
                        start=(qc == 0),
                        stop=(qc == NQC - 1),
                    )

                # O accumulation: one ht at a time so PSUM fits (4 qc tiles)
                for ht in range(2):
                    o_ps = [
                        opsum.tile([P, 512], f32, tag="op", name=f"o_{qc}")
                        for qc in range(NQC)
                    ]
                    for i, tkc in enumerate(actives):
                        for qc in range(NQC):
                            nc.tensor.matmul(
                                o_ps[qc],
                                et[:, tkc, qc * P : (qc + 1) * P],
                                V[:, tkc, ht * 512 : (ht + 1) * 512],
                                start=(i == 0),
                                stop=(i == len(actives) - 1),
                            )
                    for qc in range(NQC):
                        rec = small.tile([P, 1], f32, tag="rec")
                        nc.vector.reciprocal(rec, dt_ps[:, qc, 0:1])
                        ot = ostage.tile([P, 512], f32, tag="ot")
                        nc.vector.tensor_scalar_mul(ot, o_ps[qc], rec)
                        nc.sync.dma_start(
                            out=out_ap[
                                q0 + qc * P : q0 + (qc + 1) * P,
                                ht * 512 : (ht + 1) * 512,
                            ],
                            in_=ot,
                        )


def build_nc():
    import concourse.mybir as mybir
    import concourse.tile as tile
    from concourse import bacc

    nc = bacc.Bacc(
        "TRN2",
        target_bir_lowering=False,
        debug=False,
        num_devices=NCORES,
    )
    f32 = mybir.dt.float32
    mmdt = getattr(mybir.dt, MM_DTYPE)
    xr = nc.dram_tensor("xr", [T, C], f32, kind="ExternalInput").ap()
    wq = nc.dram_tensor("wq", [C, H], mmdt, kind="ExternalInput").ap()
    wk = nc.dram_tensor("wk", [C, H], mmdt, kind="ExternalInput").ap()
    wv = nc.dram_tensor("wv", [C, H], mmdt, kind="ExternalInput").ap()
    tailbias = nc.dram_tensor(
        "tailbias", [P, 1], f32, kind="ExternalInput"
    ).ap()
    out = nc.dram_tensor("out", [TQ, H], f32, kind="ExternalOutput").ap()
    with tile.TileContext(nc) as tc:
        _attn_body(tc, out, xr, wq, wk, wv, tailbias)
    nc.compile()
    return nc


def make_in_maps(x, Wk, Wq, Wv):
    x = np.asarray(x, dtype=np.float32)
    Wk = np.ascontiguousarray(np.asarray(Wk, dtype=np.float32))
    Wq = np.ascontiguousarray(np.asarray(Wq, dtype=np.float32))
    Wv = np.ascontiguousarray(np.asarray(Wv, dtype=np.float32))
    in_maps = []
    for c in range(NCORES):
        b, qh = divmod(c, 2)
        xb = x[b]
        if qh == 0:
            xr = np.ascontiguousarray(xb)
            tail = np.full((P, 1), TAIL_BIAS, dtype=np.float32)
        else:
            xr = np.ascontiguousarray(
                np.concatenate([xb[TQ:], xb[:TQ]], axis=0)
            )
            tail = np.zeros((P, 1), dtype=np.float32)
        in_maps.append(
            {"xr": xr, "wq": Wq, "wk": Wk, "wv": Wv, "tailbias": tail}
        )
    return in_maps


def assemble_out(results):
    out = np.empty((B, T, H), dtype=np.float32)
    for c in range(NCORES):
        b, qh = divmod(c, 2)
        out[b, qh * TQ : (qh + 1) * TQ] = results[c]["out"]
    return out


def kernel(x, Wk, Wq, Wv):
    from concourse import bass_utils

    nc = build_nc()
    in_maps = make_in_maps(x, Wk, Wq, Wv)
    res = bass_utils.run_bass_kernel_spmd(
        nc, in_maps, core_ids=list(range(NCORES))
    )
    return assemble_out(res.results)
